# revision 3
# baseline (speedup 1.0000x reference)
"""Trainium2 Bass kernel for nn_CoconAttention (dense transformer attention block).

Sharding: 8 cores = 4 batches x 2 head-groups (8 heads each). Each core gets
pre-transposed/sliced bf16 inputs, computes its partial output outT [1024, 896]
(bf16, transposed, pre-b_proj), and the host sums head-group pairs + transposes.

v2 design (all-bf16 matmul path, PE-saturating schedule):
  - every matmul operand bf16 (FWL weight loads, half the HBM traffic)
  - PE warm-up stream at t~3.5us so HAM un-throttles before real work
  - attention software-pipelined: scores(c+1) issued before PV(c) so the PE
    never waits on the exp/mask chain
  - PV psum rows leave via gpsimd cast-DMA (f32->bf16 + partition shift); the
    softmax denominator row is reciprocal'd on DVE and broadcast across
    partitions with a K=1 ones-matmul (no DRAM bounce)
  - masks multiply on the (otherwise idle) Pool engine
  - qk projections interleaved with attention pairs to fill PE gaps
"""
import os
import sys

import numpy as np
import ml_dtypes

try:
    import concourse.bass as bass
except ImportError:  # fresh grading dir: fall back to the repo location
    sys.path.insert(0, "/opt/trn_rl_repo")
    import concourse.bass as bass
import concourse.bacc as bacc

import concourse.tile as tile
from concourse import mybir
from concourse.bass_utils import run_bass_kernel_spmd
from contextlib import ExitStack

F32 = mybir.dt.float32
BF16 = mybir.dt.bfloat16
AF = mybir.ActivationFunctionType

T, Tc, NX = 896, 128, 1024
TCH = ((0, 512), (512, 896))  # tok chunks
NPAIR = 4  # head pairs per core
NWARM = 24  # PE warm-up matmuls


def _bc0(ap, n):
    """Partition-broadcast read AP: [1, ...] -> [n, ...] with partition step 0."""
    return bass.AP(tensor=ap.tensor, offset=ap.offset, ap=[[0, n]] + list(ap.ap[1:]))


def _band_pieces(c, ts, te):
    """Mask applications for chunk c in [ts,te): (s0, e0, mask_col_offset)."""
    if c == 0:
        bs, be, moff, borig = 0, 128, 128, 0  # diag half only
    elif c <= 6:
        bs = 128 * (c - 1)
        be, moff, borig = bs + 256, 0, bs  # causal(128) + diag(128)
    else:
        bs, be, moff, borig = 768, 896, 0, 768  # causal half only
    s0, e0 = max(bs, ts), min(be, te)
    if s0 >= e0:
        return []
    return [(s0, e0, moff + (s0 - borig))]


def build_nc():
    nc = bacc.Bacc("TRN2", target_bir_lowering=False)

    x_h = nc.dram_tensor("xT", [NX, T], BF16, kind="ExternalInput")
    ctx_h = nc.dram_tensor("ctxT", [NX, Tc], BF16, kind="ExternalInput")
    wq_h = nc.dram_tensor("w_q", [NX, 512], BF16, kind="ExternalInput")
    wk_h = nc.dram_tensor("w_k", [NX, 512], BF16, kind="ExternalInput")
    wv_h = nc.dram_tensor("w_v", [NX, 512], BF16, kind="ExternalInput")
    wkc_h = nc.dram_tensor("w_kc", [NX, 512], BF16, kind="ExternalInput")
    wvc_h = nc.dram_tensor("w_vc", [NX, 512], BF16, kind="ExternalInput")
    wpj_h = nc.dram_tensor("w_pj", [512, NX], BF16, kind="ExternalInput")
    bqk_h = nc.dram_tensor("b_qk", [128, 8], F32, kind="ExternalInput")
    bkc_h = nc.dram_tensor("b_kc", [128, 4], F32, kind="ExternalInput")
    bv_h = nc.dram_tensor("b_v", [1, 512], F32, kind="ExternalInput")
    bvc_h = nc.dram_tensor("b_vc", [1, 512], F32, kind="ExternalInput")
    mb_h = nc.dram_tensor("mband", [128, 256], BF16, kind="ExternalInput")
    out_h = nc.dram_tensor("outT", [NX, T], BF16, kind="ExternalOutput")

    with tile.TileContext(nc) as tc, ExitStack() as top:
        consts = top.enter_context(tc.tile_pool(name="consts", bufs=1))
        wts = top.enter_context(tc.tile_pool(name="wts", bufs=1))
        qkp = top.enter_context(tc.tile_pool(name="qkp", bufs=1))
        vtp = top.enter_context(tc.tile_pool(name="vtp", bufs=1))
        probsp = top.enter_context(tc.tile_pool(name="probsp", bufs=4))
        smallp = top.enter_context(tc.tile_pool(name="smallp", bufs=2))
        outp = top.enter_context(tc.tile_pool(name="outp", bufs=2))
        scp = top.enter_context(tc.tile_pool(name="scp", bufs=3, space="PSUM"))
        pvp = top.enter_context(tc.tile_pool(name="pvp", bufs=1, space="PSUM"))

        # ---- constants ----
        maskband = consts.tile([128, 256], BF16, name="maskband")
        nc.sync.dma_start(out=maskband, in_=mb_h[:, :])
        bias_qk = consts.tile([128, 8], F32, name="bias_qk")
        nc.sync.dma_start(out=bias_qk, in_=bqk_h[:, :])
        bias_kc = consts.tile([128, 4], F32, name="bias_kc")
        nc.sync.dma_start(out=bias_kc, in_=bkc_h[:, :])

        ebias = consts.tile([128, 2], F32, name="ebias")  # exp bias: [0]=0, [1]=ctx -2
        nc.vector.memset(ebias[:, 0:1], 0.0)
        nc.vector.memset(ebias[:, 1:2], -2.0)
        ones_sb = consts.tile([1, 64], BF16, name="ones_sb")
        nc.vector.memset(ones_sb, 1.0)
        dumm = consts.tile([128, 512], BF16, name="dumm")
        nc.vector.memset(dumm, 0.0)

        bvb = consts.tile([128, 512], F32, name="bvb")
        nc.gpsimd.dma_start(out=bvb, in_=_bc0(bv_h[:, :], 128))
        bvcb = consts.tile([128, 512], F32, name="bvcb")
        nc.gpsimd.dma_start(out=bvcb, in_=_bc0(bvc_h[:, :], 128))

        # ---- persistent activation tiles ----
        qT = [qkp.tile([128, T], BF16, name=f"qT{p}") for p in range(NPAIR)]
        kT = [qkp.tile([128, Tc + T], BF16, name=f"kT{p}") for p in range(NPAIR)]
        aT = [qkp.tile([128, T], BF16, name=f"aT{p}") for p in range(NPAIR)]
        v_sb = [vtp.tile([128, 8, 65], BF16, name=f"v{c}") for c in range(8)]
        for c in range(8):
            nc.vector.memset(v_sb[c][:, :, 64:65], 1.0)

        # ---- PE warm-up: junk matmuls so HAM un-throttles before real work ----
        warm_ps = scp.tile([128, 2, 512], F32, tag="sc", name="warm_ps")
        for i in range(NWARM):
            nc.tensor.matmul(
                warm_ps[:, i % 2, :], dumm[:, 0:128], dumm[:, :],
                start=True, stop=True, skip_group_check=True)

        # ---- input loads (small ctx operands first, then per-kc qk, then v) ----
        ctx_sb = wts.tile([128, 8, Tc], BF16, name="ctx_sb")
        nc.sync.dma_start(out=ctx_sb, in_=ctx_h[:, :].rearrange("(kc p) t -> p kc t", p=128))
        wkc_sb = wts.tile([128, 8, 512], BF16, name="wkc_sb")
        nc.sync.dma_start(out=wkc_sb, in_=wkc_h[:, :].rearrange("(kc p) f -> p kc f", p=128))
        x_sb = wts.tile([128, 8, T], BF16, name="x_sb")
        wq_sb = wts.tile([128, 8, 512], BF16, name="wq_sb")
        wk_sb = wts.tile([128, 8, 512], BF16, name="wk_sb")
        xr = x_h[:, :].rearrange("(kc p) t -> p kc t", p=128)
        qr = wq_h[:, :].rearrange("(kc p) f -> p kc f", p=128)
        kr = wk_h[:, :].rearrange("(kc p) f -> p kc f", p=128)
        for kc in range(8):
            nc.sync.dma_start(out=x_sb[:, kc, :], in_=xr[:, kc, :])
            nc.sync.dma_start(out=wq_sb[:, kc, :], in_=qr[:, kc, :])
            nc.sync.dma_start(out=wk_sb[:, kc, :], in_=kr[:, kc, :])
        wvc_sb = wts.tile([128, 8, 512], BF16, name="wvc_sb")
        nc.sync.dma_start(out=wvc_sb, in_=wvc_h[:, :].rearrange("(kc p) f -> p kc f", p=128))
        wv_sb = wts.tile([128, 8, 512], BF16, name="wv_sb")
        nc.sync.dma_start(out=wv_sb, in_=wv_h[:, :].rearrange("(kc p) f -> p kc f", p=128))
        wpj_sb = wts.tile([128, 4, 1024], BF16, name="wpj_sb")
        nc.sync.dma_start(out=wpj_sb, in_=wpj_h[:, :].rearrange("(kc p) o -> p kc o", p=128))

        # ---- emitters ----
        def emit_ctx():
            # kcT: context keys, feature-major, into kT[p][:, 0:Tc]
            for g in range(2):
                pt = scp.tile([128, 2, 512], F32, tag="sc", name=f"pkc{g}")
                for h in range(2):
                    f = 2 * g + h
                    for kc in range(8):
                        nc.tensor.matmul(
                            pt[:, h, 0:Tc], wkc_sb[:, kc, 128 * f:128 * f + 128],
                            ctx_sb[:, kc, :], start=(kc == 0), stop=(kc == 7))
                for h in range(2):
                    f = 2 * g + h
                    nc.scalar.activation(
                        out=kT[f][:, 0:Tc], in_=pt[:, h, 0:Tc], func=AF.Identity,
                        bias=bias_kc[:, f:f + 1], scale=1.0)

        def emit_qk(p):
            # qT[p] and kT[p][:, Tc:] (feature-major projections)
            for w_sb, dest, dcol, bcol in ((wq_sb, qT[p], 0, p), (wk_sb, kT[p], Tc, 4 + p)):
                pt = scp.tile([128, 2, 512], F32, tag="sc", name=f"pqk{p}{dcol}")
                for ti, (ts, te) in enumerate(TCH):
                    for kc in range(8):
                        nc.tensor.matmul(
                            pt[:, ti, 0:te - ts], w_sb[:, kc, 128 * p:128 * p + 128],
                            x_sb[:, kc, ts:te], start=(kc == 0), stop=(kc == 7))
                for ti, (ts, te) in enumerate(TCH):
                    nc.scalar.activation(
                        out=dest[:, dcol + ts:dcol + te], in_=pt[:, ti, 0:te - ts],
                        func=AF.Identity, bias=bias_qk[:, bcol:bcol + 1], scale=1.0)

        def emit_vc():
            pt = scp.tile([128, 2, 512], F32, tag="sc", name="pvc")
            for kc in range(8):
                nc.tensor.matmul(
                    pt[:, 0, :], ctx_sb[:, kc, :], wvc_sb[:, kc, :],
                    start=(kc == 0), stop=(kc == 7))
            nc.vector.tensor_add(
                out=v_sb[0][:, :, 0:64],
                in0=pt[:, 0, :].rearrange("p (h d) -> p h d", h=8),
                in1=bvcb.rearrange("p (h d) -> p h d", h=8))

        def emit_v():
            # v natural layout [tok-chunk, head, 64] (+ ones col for denominator)
            for g in range(4):
                tts = [tt for tt in (2 * g, 2 * g + 1) if tt < 7]
                pt = scp.tile([128, 2, 512], F32, tag="sc", name=f"pv{g}")
                for h, tt in enumerate(tts):
                    for kc in range(8):
                        nc.tensor.matmul(
                            pt[:, h, :], x_sb[:, kc, 128 * tt:128 * tt + 128],
                            wv_sb[:, kc, :], start=(kc == 0), stop=(kc == 7))
                for h, tt in enumerate(tts):
                    nc.vector.tensor_add(
                        out=v_sb[1 + tt][:, :, 0:64],
                        in0=pt[:, h, :].rearrange("p (h d) -> p h d", h=8),
                        in1=bvb.rearrange("p (h d) -> p h d", h=8))

        deferred = []  # normalize closures, delayed so PE isn't head-blocked

        def emit_att(p, ti):
            ts, te = TCH[ti]
            w = te - ts
            lives = [c for c in range(8) if max(128 * (c - 1), ts) < te]
            at = pvp.tile([65, 2, 512], F32, tag="pv", name=f"at{p}{ti}")
            pbs = {}

            def S(c):
                cs = max(128 * (c - 1), ts)
                sc = scp.tile([128, 2, 512], F32, tag="sc", name=f"sc{p}{ti}{c}")
                for hi in range(2):
                    nc.tensor.matmul(
                        sc[:, hi, cs - ts:w],
                        kT[p][64 * hi:64 * hi + 64, 128 * c:128 * c + 128],
                        qT[p][64 * hi:64 * hi + 64, cs:te],
                        start=True, stop=True, tile_position=(64 * hi, 0))
                pb = probsp.tile([128, 2, 512], BF16, tag="pb", name=f"pb{p}{ti}{c}")
                nc.scalar.activation(
                    out=pb[:, :, cs - ts:w], in_=sc[:, :, cs - ts:w], func=AF.Exp,
                    bias=(ebias[:, 1:2] if c == 0 else ebias[:, 0:1]), scale=0.125)
                for hi in range(2):
                    for s0, e0, mc in _band_pieces(c, ts, te):
                        nc.gpsimd.tensor_mul(
                            out=pb[:, hi, s0 - ts:e0 - ts],
                            in0=pb[:, hi, s0 - ts:e0 - ts],
                            in1=maskband[:, mc:mc + (e0 - s0)])
                pbs[c] = (pb, cs)

            def P(c):
                pb, cs = pbs.pop(c)
                for hi in range(2):
                    nc.tensor.matmul(
                        at[0:65, hi, cs - ts:w], v_sb[c][:, 2 * p + hi, :],
                        pb[:, hi, cs - ts:w],
                        start=(c == lives[0]), stop=(c == lives[-1]),
                        skip_group_check=True)

            S(lives[0])
            S(lives[1])
            if deferred:
                deferred.pop(0)()
            P(lives[0])
            for i in range(2, len(lives)):
                S(lives[i])
                P(lives[i - 1])
            P(lives[-1])

            # evacuate PV psum: cast to bf16 staging (DVE), shift to aT via DMA
            stg = smallp.tile([65, 2, 512], BF16, tag="stg", name=f"stg{p}{ti}")
            nc.vector.tensor_copy(out=stg[:, :, 0:w], in_=at[:, :, 0:w])
            nc.sync.dma_start(out=aT[p][0:64, ts:te], in_=stg[0:64, 0, 0:w])
            nc.sync.dma_start(out=aT[p][64:128, ts:te], in_=stg[0:64, 1, 0:w])
            rd = smallp.tile([1, 2, 512], BF16, tag="rd", name=f"rd{p}{ti}")
            with nc.allow_low_precision(reason="softmax denom reciprocal in bf16"):
                nc.vector.reciprocal(out=rd[:, :, 0:w], in_=stg[64:65, :, 0:w])

            def finish():
                rb = scp.tile([128, 2, 512], F32, tag="sc", name=f"rb{p}{ti}")
                nc.tensor.matmul(
                    rb[0:64, 0, 0:w], ones_sb[0:1, :], rd[0:1, 0, 0:w],
                    start=True, stop=True, tile_position=(0, 0),
                    skip_group_check=True)
                nc.tensor.matmul(
                    rb[64:128, 0, 0:w], ones_sb[0:1, :], rd[0:1, 1, 0:w],
                    start=True, stop=True, tile_position=(0, 64),
                    skip_group_check=True)
                nc.vector.tensor_mul(
                    out=aT[p][:, ts:te], in0=aT[p][:, ts:te], in1=rb[:, 0, 0:w])

            deferred.append(finish)

        def emit_out():
            while deferred:
                deferred.pop(0)()
            for of in range(8):
                pt = scp.tile([128, 2, 512], F32, tag="sc", name=f"po{of}")
                for ti, (ts, te) in enumerate(TCH):
                    for kc in range(4):
                        nc.tensor.matmul(
                            pt[:, ti, 0:te - ts], wpj_sb[:, kc, 128 * of:128 * of + 128],
                            aT[kc][:, ts:te], start=(kc == 0), stop=(kc == 3))
                ob = outp.tile([128, T], BF16, tag="ob", name=f"ob{of}")
                for ti, (ts, te) in enumerate(TCH):
                    nc.scalar.copy(out=ob[:, ts:te], in_=pt[:, ti, 0:te - ts])
                nc.sync.dma_start(out=out_h[128 * of:128 * of + 128, :], in_=ob)

        # ---- master schedule: projections interleaved with attention ----
        emit_ctx()
        emit_qk(0)
        emit_vc()
        emit_v()
        emit_qk(1)
        emit_att(0, 0)
        emit_qk(2)
        emit_att(0, 1)
        emit_att(1, 0)
        emit_qk(3)
        emit_att(1, 1)
        emit_att(2, 0)
        emit_att(2, 1)
        emit_att(3, 0)
        emit_att(3, 1)
        emit_out()

    if not nc.is_finalized():
        nc.finalize()
    return nc


_NC_CACHE = {}


def _get_nc():
    if "nc" not in _NC_CACHE:
        _NC_CACHE["nc"] = build_nc()
    return _NC_CACHE["nc"]


def _pack128(v):
    """[128*n] -> [128, n] with [p, f] = v[128*f + p]."""
    n = v.shape[0] // 128
    return np.ascontiguousarray(v.reshape(n, 128).T)


def make_in_maps(inputs):
    bf16 = ml_dtypes.bfloat16
    x = np.asarray(inputs["x"], np.float32)
    ctx_seq = np.asarray(inputs["context_seq"], np.float32)
    w_ref = np.asarray(inputs["w_ref"], np.float32)
    b_ref = np.asarray(inputs["b_ref"], np.float32)
    w_attn = np.asarray(inputs["w_attn"], np.float32)
    b_attn = np.asarray(inputs["b_attn"], np.float32)
    w_proj = np.asarray(inputs["w_proj"], np.float32)

    # mask band constant: cols 0-127 causal (1 where q>=p), cols 128-255
    # anti-diagonal (0 where q==p else 1)
    qq = np.arange(128)[None, :]
    pp = np.arange(128)[:, None]
    mband = np.concatenate([(qq >= pp), (qq != pp)], axis=1).astype(bf16)
    mband = np.ascontiguousarray(mband)

    in_maps = []
    for b in range(4):
        xT = np.ascontiguousarray(x[b].T.astype(bf16))
        ctxT = np.ascontiguousarray(ctx_seq[b].T.astype(bf16))
        for g in range(2):
            sl = slice(512 * g, 512 * g + 512)
            in_maps.append(dict(
                xT=xT,
                ctxT=ctxT,
                w_q=np.ascontiguousarray(w_attn[:, 0 * NX:1 * NX][:, sl].astype(bf16)),
                w_k=np.ascontiguousarray(w_attn[:, 1 * NX:2 * NX][:, sl].astype(bf16)),
                w_v=np.ascontiguousarray(w_attn[:, 2 * NX:3 * NX][:, sl].astype(bf16)),
                w_kc=np.ascontiguousarray(w_ref[:, 0 * NX:1 * NX][:, sl].astype(bf16)),
                w_vc=np.ascontiguousarray(w_ref[:, 1 * NX:2 * NX][:, sl].astype(bf16)),
                w_pj=np.ascontiguousarray(w_proj[sl, :].astype(bf16)),
                b_qk=_pack128(np.concatenate([b_attn[0 * NX:1 * NX][sl],
                                              b_attn[1 * NX:2 * NX][sl]])),
                b_kc=_pack128(b_ref[0 * NX:1 * NX][sl]),
                b_v=np.ascontiguousarray(b_attn[2 * NX:3 * NX][sl].reshape(1, 512)),
                b_vc=np.ascontiguousarray(b_ref[1 * NX:2 * NX][sl].reshape(1, 512)),
                mband=mband,
            ))
    return in_maps


def kernel(**inputs):
    b_proj = np.asarray(inputs["b_proj"], np.float32)
    in_maps = make_in_maps(inputs)
    nc = _get_nc()
    res = run_bass_kernel_spmd(nc, in_maps, core_ids=list(range(8)),
                               trace=os.environ.get("COCON_TRACE", "") == "1")
    outs = res.results
    out = np.empty((4, T, NX), np.float32)
    for b in range(4):
        acc = outs[2 * b]["outT"].astype(np.float32) + \
            outs[2 * b + 1]["outT"].astype(np.float32)  # [1024, 896]
        out[b] = acc.T + b_proj[None, :]
    if res.exec_time_ns is not None:
        kernel.last_exec_time_ns = res.exec_time_ns
    return out


kernel.last_exec_time_ns = None


# revision 9
# speedup vs baseline: 1.2927x; 1.2927x over previous
"""Trainium2 Bass kernel for nn_CoconAttention (dense transformer attention block).

Sharding: 8 cores = 4 batches x 2 head-groups (8 heads each). Each core gets
pre-transposed/sliced bf16 inputs, computes its partial output outT [1024, 896]
(bf16, transposed, pre-b_proj), and the host sums head-group pairs + transposes.

v3 design (all-bf16 matmul path, gap-filled PE schedule):
  - every matmul operand bf16 (FWL weight loads, half the HBM traffic)
  - loads spread across the three DMA queues (sync/scalar HW-DGE + gpsimd)
  - PE warm-up stream so HAM un-throttles before real work
  - attention software-pipelined: scores run 2-3 chunks ahead of PV, and
    projection / out-proj / normalize psum-groups are popped from a filler
    queue into the gaps so the in-order PE never waits on the exp/mask chain
  - PV psum leaves via DVE cast-copy then a plain DMA partition-shift; the
    denominator row is reciprocal'd (fast approx) and broadcast across
    partitions with a K=1 ones-matmul (no DRAM bounce)
  - masks: hi0 on DVE, hi1 on Pool (parallel engines)
"""
import os
import sys

import numpy as np
import ml_dtypes

try:
    import concourse.bass as bass
except ImportError:  # fresh grading dir: fall back to the repo location
    sys.path.insert(0, "/opt/trn_rl_repo")
    import concourse.bass as bass
import concourse.bacc as bacc

import concourse.tile as tile
from concourse import mybir
from concourse.bass_utils import run_bass_kernel_spmd
from contextlib import ExitStack

F32 = mybir.dt.float32
BF16 = mybir.dt.bfloat16
AF = mybir.ActivationFunctionType

T, Tc, NX = 896, 128, 1024
TCH = ((0, 512), (512, 896))  # tok chunks
NPAIR = 4  # head pairs per core
NWARM = 16  # PE warm-up matmuls


def _bc0(ap, n):
    """Partition-broadcast read AP: [1, ...] -> [n, ...] with partition step 0."""
    return bass.AP(tensor=ap.tensor, offset=ap.offset, ap=[[0, n]] + list(ap.ap[1:]))


def _band_pieces(c, ts, te):
    """Mask applications for chunk c in [ts,te): (s0, e0, mask_col_offset)."""
    if c == 0:
        bs, be, moff, borig = 0, 128, 128, 0  # diag half only
    elif c <= 6:
        bs = 128 * (c - 1)
        be, moff, borig = bs + 256, 0, bs  # causal(128) + diag(128)
    else:
        bs, be, moff, borig = 768, 896, 0, 768  # causal half only
    s0, e0 = max(bs, ts), min(be, te)
    if s0 >= e0:
        return []
    return [(s0, e0, moff + (s0 - borig))]


def build_nc():
    nc = bacc.Bacc("TRN2", target_bir_lowering=False)

    x_h = nc.dram_tensor("xT", [NX, T], BF16, kind="ExternalInput")
    ctx_h = nc.dram_tensor("ctxT", [NX, Tc], BF16, kind="ExternalInput")
    wq_h = nc.dram_tensor("w_q", [NX, 512], BF16, kind="ExternalInput")
    wk_h = nc.dram_tensor("w_k", [NX, 512], BF16, kind="ExternalInput")
    wv_h = nc.dram_tensor("w_v", [NX, 512], BF16, kind="ExternalInput")
    wkc_h = nc.dram_tensor("w_kc", [NX, 512], BF16, kind="ExternalInput")
    wvc_h = nc.dram_tensor("w_vc", [NX, 512], BF16, kind="ExternalInput")
    wpj_h = nc.dram_tensor("w_pj", [512, NX], BF16, kind="ExternalInput")
    bqk_h = nc.dram_tensor("b_qk", [128, 8], F32, kind="ExternalInput")
    bkc_h = nc.dram_tensor("b_kc", [128, 4], F32, kind="ExternalInput")
    bv_h = nc.dram_tensor("b_v", [1, 512], F32, kind="ExternalInput")
    bvc_h = nc.dram_tensor("b_vc", [1, 512], F32, kind="ExternalInput")
    mb_h = nc.dram_tensor("mband", [128, 256], BF16, kind="ExternalInput")
    out_h = nc.dram_tensor("outT", [NX, T], BF16, kind="ExternalOutput")

    with tile.TileContext(nc) as tc, ExitStack() as top:
        consts = top.enter_context(tc.tile_pool(name="consts", bufs=1))
        wts = top.enter_context(tc.tile_pool(name="wts", bufs=1))
        qkp = top.enter_context(tc.tile_pool(name="qkp", bufs=1))
        vtp = top.enter_context(tc.tile_pool(name="vtp", bufs=1))
        probsp = top.enter_context(tc.tile_pool(name="probsp", bufs=4))
        smallp = top.enter_context(tc.tile_pool(name="smallp", bufs=2))
        scp = top.enter_context(tc.tile_pool(name="scp", bufs=3, space="PSUM"))
        pvp = top.enter_context(tc.tile_pool(name="pvp", bufs=1, space="PSUM"))

        # ---- constants (sync queue, tiny) ----
        maskband = consts.tile([128, 256], BF16, name="maskband")
        nc.sync.dma_start(out=maskband, in_=mb_h[:, :])
        bias_qk = consts.tile([128, 8], F32, name="bias_qk")
        nc.sync.dma_start(out=bias_qk, in_=bqk_h[:, :])
        bias_kc = consts.tile([128, 4], F32, name="bias_kc")
        nc.sync.dma_start(out=bias_kc, in_=bkc_h[:, :])

        ebias = consts.tile([128, 2], F32, name="ebias")  # exp bias: [0]=0, [1]=ctx -2
        nc.vector.memset(ebias[:, 0:1], 0.0)
        nc.vector.memset(ebias[:, 1:2], -2.0)
        # ones row lives at partition 64 to match the PV-psum denominator row
        # (matmul fmap and weight must start at the same partition index)
        ones_sb = consts.tile([65, 64], BF16, name="ones_sb")
        nc.vector.memset(ones_sb[64:65, :], 1.0)
        dumm = consts.tile([128, 512], BF16, name="dumm")
        nc.vector.memset(dumm, 0.0)

        # ---- persistent activation tiles ----
        qT = [qkp.tile([128, T], BF16, name=f"qT{p}") for p in range(NPAIR)]
        kT = [qkp.tile([128, Tc + T], BF16, name=f"kT{p}") for p in range(NPAIR)]
        aT = [qkp.tile([128, T], BF16, name=f"aT{p}") for p in range(NPAIR)]
        obuf = qkp.tile([128, 8, T], BF16, name="obuf")
        v_sb = [vtp.tile([128, 8, 65], BF16, name=f"v{c}") for c in range(8)]
        for c in range(8):
            nc.vector.memset(v_sb[c][:, :, 64:65], 1.0)

        # ---- PE warm-up: junk matmuls so HAM un-throttles before real work ----
        warm_ps = scp.tile([128, 2, 512], F32, tag="mm", name="warm_ps")
        for i in range(NWARM):
            nc.tensor.matmul(
                warm_ps[:, i % 2, :], dumm[:, 0:128], dumm[:, :],
                start=True, stop=True, skip_group_check=True)

        # ---- input loads, spread across the three DMA queues ----
        ctx_sb = wts.tile([128, 8, Tc], BF16, name="ctx_sb")
        nc.sync.dma_start(out=ctx_sb, in_=ctx_h[:, :].rearrange("(kc p) t -> p kc t", p=128))
        wkc_sb = wts.tile([128, 8, 512], BF16, name="wkc_sb")
        nc.sync.dma_start(out=wkc_sb, in_=wkc_h[:, :].rearrange("(kc p) f -> p kc f", p=128))
        wq_sb = wts.tile([128, 8, 512], BF16, name="wq_sb")
        nc.sync.dma_start(out=wq_sb, in_=wq_h[:, :].rearrange("(kc p) f -> p kc f", p=128))
        wvc_sb = wts.tile([128, 8, 512], BF16, name="wvc_sb")
        nc.sync.dma_start(out=wvc_sb, in_=wvc_h[:, :].rearrange("(kc p) f -> p kc f", p=128))

        x_sb = wts.tile([128, 8, T], BF16, name="x_sb")
        nc.scalar.dma_start(out=x_sb, in_=x_h[:, :].rearrange("(kc p) t -> p kc t", p=128))
        wk_sb = wts.tile([128, 8, 512], BF16, name="wk_sb")
        nc.scalar.dma_start(out=wk_sb, in_=wk_h[:, :].rearrange("(kc p) f -> p kc f", p=128))

        bvb = consts.tile([128, 512], F32, name="bvb")
        nc.gpsimd.dma_start(out=bvb, in_=_bc0(bv_h[:, :], 128))
        bvcb = consts.tile([128, 512], F32, name="bvcb")
        nc.gpsimd.dma_start(out=bvcb, in_=_bc0(bvc_h[:, :], 128))
        wv_sb = wts.tile([128, 8, 512], BF16, name="wv_sb")
        nc.gpsimd.dma_start(out=wv_sb, in_=wv_h[:, :].rearrange("(kc p) f -> p kc f", p=128))
        wpj_sb = wts.tile([128, 4, 1024], BF16, name="wpj_sb")
        nc.gpsimd.dma_start(out=wpj_sb, in_=wpj_h[:, :].rearrange("(kc p) o -> p kc o", p=128))

        # ---- psum-group emitters (each allocates one "mm" tile) ----
        def emit_ctx():
            # kcT: context keys, feature-major, into kT[p][:, 0:Tc]
            for g in range(2):
                pt = scp.tile([128, 2, 512], F32, tag="mm", name=f"pkc{g}")
                for h in range(2):
                    f = 2 * g + h
                    for kc in range(8):
                        nc.tensor.matmul(
                            pt[:, h, 0:Tc], wkc_sb[:, kc, 128 * f:128 * f + 128],
                            ctx_sb[:, kc, :], start=(kc == 0), stop=(kc == 7))
                for h in range(2):
                    f = 2 * g + h
                    nc.scalar.activation(
                        out=kT[f][:, 0:Tc], in_=pt[:, h, 0:Tc], func=AF.Identity,
                        bias=bias_kc[:, f:f + 1], scale=1.0)

        def qk_filler(p, which, ti):
            w_sb = wq_sb if which == 0 else wk_sb
            dest = qT[p] if which == 0 else kT[p]
            dcol = 0 if which == 0 else Tc
            bcol = p if which == 0 else 4 + p
            ts, te = TCH[ti]

            def f():
                pt = scp.tile([128, 2, 512], F32, tag="mm", name=f"pqk{p}{which}{ti}")
                for kc in range(8):
                    nc.tensor.matmul(
                        pt[:, 0, 0:te - ts], w_sb[:, kc, 128 * p:128 * p + 128],
                        x_sb[:, kc, ts:te], start=(kc == 0), stop=(kc == 7))
                nc.scalar.activation(
                    out=dest[:, dcol + ts:dcol + te], in_=pt[:, 0, 0:te - ts],
                    func=AF.Identity, bias=bias_qk[:, bcol:bcol + 1], scale=1.0)
            return f

        def emit_vc():
            pt = scp.tile([128, 2, 512], F32, tag="mm", name="pvc")
            for kc in range(8):
                nc.tensor.matmul(
                    pt[:, 0, :], ctx_sb[:, kc, :], wvc_sb[:, kc, :],
                    start=(kc == 0), stop=(kc == 7))
            nc.vector.tensor_add(
                out=v_sb[0][:, :, 0:64],
                in0=pt[:, 0, :].rearrange("p (h d) -> p h d", h=8),
                in1=bvcb.rearrange("p (h d) -> p h d", h=8))

        def emit_v():
            # v natural layout [tok-chunk, head, 64] (+ ones col for denominator)
            for g in range(4):
                tts = [tt for tt in (2 * g, 2 * g + 1) if tt < 7]
                pt = scp.tile([128, 2, 512], F32, tag="mm", name=f"pv{g}")
                for h, tt in enumerate(tts):
                    for kc in range(8):
                        nc.tensor.matmul(
                            pt[:, h, :], x_sb[:, kc, 128 * tt:128 * tt + 128],
                            wv_sb[:, kc, :], start=(kc == 0), stop=(kc == 7))
                for h, tt in enumerate(tts):
                    nc.vector.tensor_add(
                        out=v_sb[1 + tt][:, :, 0:64],
                        in0=pt[:, h, :].rearrange("p (h d) -> p h d", h=8),
                        in1=bvb.rearrange("p (h d) -> p h d", h=8))

        def out_filler(of, ti):
            ts, te = TCH[ti]

            def f():
                pt = scp.tile([128, 2, 512], F32, tag="mm", name=f"po{of}{ti}")
                for kc in range(4):
                    nc.tensor.matmul(
                        pt[:, 0, 0:te - ts], wpj_sb[:, kc, 128 * of:128 * of + 128],
                        aT[kc][:, ts:te], start=(kc == 0), stop=(kc == 3))
                nc.scalar.copy(out=obuf[:, of, ts:te], in_=pt[:, 0, 0:te - ts])
                if ti == 1:
                    nc.sync.dma_start(
                        out=out_h[128 * of:128 * of + 128, :], in_=obuf[:, of, :])
            return f

        fillers = []

        def emit_att(p, ti):
            ts, te = TCH[ti]
            w = te - ts
            lives = [c for c in range(8) if max(128 * (c - 1), ts) < te]
            n = len(lives)
            at = pvp.tile([65, 2, 512], F32, tag="pv", name=f"at{p}{ti}")
            pbs = {}

            def S(c):
                cs = max(128 * (c - 1), ts)
                sc = scp.tile([128, 2, 512], F32, tag="mm", name=f"sc{p}{ti}{c}")
                for hi in range(2):
                    nc.tensor.matmul(
                        sc[:, hi, cs - ts:w],
                        kT[p][64 * hi:64 * hi + 64, 128 * c:128 * c + 128],
                        qT[p][64 * hi:64 * hi + 64, cs:te],
                        start=True, stop=True, tile_position=(64 * hi, 0))
                pb = probsp.tile([128, 2, 512], BF16, tag="pb", name=f"pb{p}{ti}{c}")
                for hi in range(2):
                    nc.scalar.activation(
                        out=pb[:, hi, cs - ts:w], in_=sc[:, hi, cs - ts:w], func=AF.Exp,
                        bias=(ebias[:, 1:2] if c == 0 else ebias[:, 0:1]), scale=0.125)
                    mask_eng = nc.vector if hi == 0 else nc.gpsimd
                    for s0, e0, mc in _band_pieces(c, ts, te):
                        mask_eng.tensor_mul(
                            out=pb[:, hi, s0 - ts:e0 - ts],
                            in0=pb[:, hi, s0 - ts:e0 - ts],
                            in1=maskband[:, mc:mc + (e0 - s0)])
                pbs[c] = (pb, cs)

            def P(c):
                pb, cs = pbs.pop(c)
                for hi in range(2):
                    nc.tensor.matmul(
                        at[0:65, hi, cs - ts:w], v_sb[c][:, 2 * p + hi, :],
                        pb[:, hi, cs - ts:w],
                        start=(c == lives[0]), stop=(c == lives[-1]),
                        skip_group_check=True)

            si = 0

            def pushS():
                nonlocal si
                if si < n:
                    S(lives[si])
                    si += 1

            pushS()
            pushS()
            for pi in range(n):
                if fillers and pi >= 1:
                    fillers.pop(0)()
                elif si - pi < 4:
                    pushS()  # no filler: deepen score-ahead instead (cap 3)
                pushS()
                P(lives[pi])

            # evacuate PV psum: cast to bf16 staging (DVE), shift to aT via DMA
            stg = smallp.tile([65, 2, 512], BF16, tag="stg", name=f"stg{p}{ti}")
            nc.vector.tensor_copy(out=stg[:, :, 0:w], in_=at[:, :, 0:w])
            nc.sync.dma_start(out=aT[p][0:64, ts:te], in_=stg[0:64, 0, 0:w])
            nc.sync.dma_start(out=aT[p][64:128, ts:te], in_=stg[0:64, 1, 0:w])

            def finish():
                # broadcast raw denom row across partitions (K=1 ones-matmul),
                # then 128-lane-parallel fast reciprocal, then normalize aT
                rb = scp.tile([128, 2, 512], F32, tag="mm", name=f"rb{p}{ti}")
                nc.tensor.matmul(
                    rb[0:64, 0, 0:w], ones_sb[64:65, :], stg[64:65, 0, 0:w],
                    start=True, stop=True, tile_position=(64, 0),
                    skip_group_check=True)
                nc.tensor.matmul(
                    rb[64:128, 0, 0:w], ones_sb[64:65, :], stg[64:65, 1, 0:w],
                    start=True, stop=True, tile_position=(64, 64),
                    skip_group_check=True)
                rbc = smallp.tile([128, 512], F32, tag="rbc", name=f"rbc{p}{ti}")
                nc.vector.reciprocal_approx_fast(out=rbc[:, 0:w], in_=rb[:, 0, 0:w])
                nc.vector.tensor_mul(
                    out=aT[p][:, ts:te], in0=aT[p][:, ts:te], in1=rbc[:, 0:w])

            fillers.insert(0, finish)

        # ---- master schedule ----
        emit_ctx()
        for which in range(2):
            for ti in range(2):
                qk_filler(0, which, ti)()
        emit_vc()
        emit_v()
        for p in range(1, NPAIR):
            for which in range(2):
                for ti in range(2):
                    fillers.append(qk_filler(p, which, ti))

        emit_att(0, 0)
        emit_att(0, 1)
        emit_att(1, 0)
        emit_att(1, 1)
        emit_att(2, 0)
        emit_att(2, 1)
        emit_att(3, 0)
        # att(3,0)'s finish (normalizes aT[3][:, t0]) must precede out-t0 reads
        fillers.extend(out_filler(of, 0) for of in range(8))
        emit_att(3, 1)
        fillers.extend(out_filler(of, 1) for of in range(8))
        while fillers:
            fillers.pop(0)()

    if not nc.is_finalized():
        nc.finalize()
    return nc


_NC_CACHE = {}


def _get_nc():
    if "nc" not in _NC_CACHE:
        _NC_CACHE["nc"] = build_nc()
    return _NC_CACHE["nc"]


def _pack128(v):
    """[128*n] -> [128, n] with [p, f] = v[128*f + p]."""
    n = v.shape[0] // 128
    return np.ascontiguousarray(v.reshape(n, 128).T)


def make_in_maps(inputs):
    bf16 = ml_dtypes.bfloat16
    x = np.asarray(inputs["x"], np.float32)
    ctx_seq = np.asarray(inputs["context_seq"], np.float32)
    w_ref = np.asarray(inputs["w_ref"], np.float32)
    b_ref = np.asarray(inputs["b_ref"], np.float32)
    w_attn = np.asarray(inputs["w_attn"], np.float32)
    b_attn = np.asarray(inputs["b_attn"], np.float32)
    w_proj = np.asarray(inputs["w_proj"], np.float32)

    # mask band constant: cols 0-127 causal (1 where q>=p), cols 128-255
    # anti-diagonal (0 where q==p else 1)
    qq = np.arange(128)[None, :]
    pp = np.arange(128)[:, None]
    mband = np.concatenate([(qq >= pp), (qq != pp)], axis=1).astype(bf16)
    mband = np.ascontiguousarray(mband)

    in_maps = []
    for b in range(4):
        xT = np.ascontiguousarray(x[b].T.astype(bf16))
        ctxT = np.ascontiguousarray(ctx_seq[b].T.astype(bf16))
        for g in range(2):
            sl = slice(512 * g, 512 * g + 512)
            in_maps.append(dict(
                xT=xT,
                ctxT=ctxT,
                w_q=np.ascontiguousarray(w_attn[:, 0 * NX:1 * NX][:, sl].astype(bf16)),
                w_k=np.ascontiguousarray(w_attn[:, 1 * NX:2 * NX][:, sl].astype(bf16)),
                w_v=np.ascontiguousarray(w_attn[:, 2 * NX:3 * NX][:, sl].astype(bf16)),
                w_kc=np.ascontiguousarray(w_ref[:, 0 * NX:1 * NX][:, sl].astype(bf16)),
                w_vc=np.ascontiguousarray(w_ref[:, 1 * NX:2 * NX][:, sl].astype(bf16)),
                w_pj=np.ascontiguousarray(w_proj[sl, :].astype(bf16)),
                b_qk=_pack128(np.concatenate([b_attn[0 * NX:1 * NX][sl],
                                              b_attn[1 * NX:2 * NX][sl]])),
                b_kc=_pack128(b_ref[0 * NX:1 * NX][sl]),
                b_v=np.ascontiguousarray(b_attn[2 * NX:3 * NX][sl].reshape(1, 512)),
                b_vc=np.ascontiguousarray(b_ref[1 * NX:2 * NX][sl].reshape(1, 512)),
                mband=mband,
            ))
    return in_maps


def kernel(**inputs):
    b_proj = np.asarray(inputs["b_proj"], np.float32)
    in_maps = make_in_maps(inputs)
    nc = _get_nc()
    res = run_bass_kernel_spmd(nc, in_maps, core_ids=list(range(8)),
                               trace=os.environ.get("COCON_TRACE", "") == "1")
    outs = res.results
    out = np.empty((4, T, NX), np.float32)
    for b in range(4):
        acc = outs[2 * b]["outT"].astype(np.float32) + \
            outs[2 * b + 1]["outT"].astype(np.float32)  # [1024, 896]
        out[b] = acc.T + b_proj[None, :]
    if res.exec_time_ns is not None:
        kernel.last_exec_time_ns = res.exec_time_ns
    return out


kernel.last_exec_time_ns = None


# revision 10
# speedup vs baseline: 1.3116x; 1.0146x over previous
"""Trainium2 Bass kernel for nn_CoconAttention (dense transformer attention block).

Sharding: 8 cores = 4 batches x 2 head-groups (8 heads each). Each core gets
pre-transposed/sliced bf16 inputs, computes its partial output outT [1024, 896]
(bf16, transposed, pre-b_proj), and the host sums head-group pairs + transposes.

v3 design (all-bf16 matmul path, gap-filled PE schedule):
  - every matmul operand bf16 (FWL weight loads, half the HBM traffic)
  - loads spread across the three DMA queues (sync/scalar HW-DGE + gpsimd)
  - PE warm-up stream so HAM un-throttles before real work
  - attention software-pipelined: scores run 2-3 chunks ahead of PV, and
    projection / out-proj / normalize psum-groups are popped from a filler
    queue into the gaps so the in-order PE never waits on the exp/mask chain
  - PV psum leaves via DVE cast-copy then a plain DMA partition-shift; the
    denominator row is reciprocal'd (fast approx) and broadcast across
    partitions with a K=1 ones-matmul (no DRAM bounce)
  - masks: hi0 on DVE, hi1 on Pool (parallel engines)
"""
import os
import sys

import numpy as np
import ml_dtypes

try:
    import concourse.bass as bass
except ImportError:  # fresh grading dir: fall back to the repo location
    sys.path.insert(0, "/opt/trn_rl_repo")
    import concourse.bass as bass
import concourse.bacc as bacc

import concourse.tile as tile
from concourse import mybir
from concourse.bass_utils import run_bass_kernel_spmd
from contextlib import ExitStack

F32 = mybir.dt.float32
BF16 = mybir.dt.bfloat16
AF = mybir.ActivationFunctionType

T, Tc, NX = 896, 128, 1024
TCH = ((0, 512), (512, 896))  # tok chunks
NPAIR = 4  # head pairs per core
NWARM = 36  # PE warm-up matmuls


def _bc0(ap, n):
    """Partition-broadcast read AP: [1, ...] -> [n, ...] with partition step 0."""
    return bass.AP(tensor=ap.tensor, offset=ap.offset, ap=[[0, n]] + list(ap.ap[1:]))


def _band_pieces(c, ts, te):
    """Mask applications for chunk c in [ts,te): (s0, e0, mask_col_offset)."""
    if c == 0:
        bs, be, moff, borig = 0, 128, 128, 0  # diag half only
    elif c <= 6:
        bs = 128 * (c - 1)
        be, moff, borig = bs + 256, 0, bs  # causal(128) + diag(128)
    else:
        bs, be, moff, borig = 768, 896, 0, 768  # causal half only
    s0, e0 = max(bs, ts), min(be, te)
    if s0 >= e0:
        return []
    return [(s0, e0, moff + (s0 - borig))]


def build_nc():
    nc = bacc.Bacc("TRN2", target_bir_lowering=False)

    x_h = nc.dram_tensor("xT", [NX, T], BF16, kind="ExternalInput")
    ctx_h = nc.dram_tensor("ctxT", [NX, Tc], BF16, kind="ExternalInput")
    wq_h = nc.dram_tensor("w_q", [NX, 512], BF16, kind="ExternalInput")
    wk_h = nc.dram_tensor("w_k", [NX, 512], BF16, kind="ExternalInput")
    wv_h = nc.dram_tensor("w_v", [NX, 512], BF16, kind="ExternalInput")
    wkc_h = nc.dram_tensor("w_kc", [NX, 512], BF16, kind="ExternalInput")
    wvc_h = nc.dram_tensor("w_vc", [NX, 512], BF16, kind="ExternalInput")
    wpj_h = nc.dram_tensor("w_pj", [512, NX], BF16, kind="ExternalInput")
    bqk_h = nc.dram_tensor("b_qk", [128, 8], F32, kind="ExternalInput")
    bkc_h = nc.dram_tensor("b_kc", [128, 4], F32, kind="ExternalInput")
    bv_h = nc.dram_tensor("b_v", [1, 512], F32, kind="ExternalInput")
    bvc_h = nc.dram_tensor("b_vc", [1, 512], F32, kind="ExternalInput")
    mb_h = nc.dram_tensor("mband", [128, 256], BF16, kind="ExternalInput")
    out_h = nc.dram_tensor("outT", [NX, T], BF16, kind="ExternalOutput")

    with tile.TileContext(nc) as tc, ExitStack() as top:
        consts = top.enter_context(tc.tile_pool(name="consts", bufs=1))
        wts = top.enter_context(tc.tile_pool(name="wts", bufs=1))
        qkp = top.enter_context(tc.tile_pool(name="qkp", bufs=1))
        vtp = top.enter_context(tc.tile_pool(name="vtp", bufs=1))
        probsp = top.enter_context(tc.tile_pool(name="probsp", bufs=4))
        smallp = top.enter_context(tc.tile_pool(name="smallp", bufs=2))
        scp = top.enter_context(tc.tile_pool(name="scp", bufs=3, space="PSUM"))
        pvp = top.enter_context(tc.tile_pool(name="pvp", bufs=1, space="PSUM"))

        # ---- constants (sync queue, tiny) ----
        maskband = consts.tile([128, 256], BF16, name="maskband")
        nc.gpsimd.dma_start(out=maskband, in_=mb_h[:, :])
        bias_qk = consts.tile([128, 8], F32, name="bias_qk")
        nc.sync.dma_start(out=bias_qk, in_=bqk_h[:, :])
        bias_kc = consts.tile([128, 4], F32, name="bias_kc")
        nc.sync.dma_start(out=bias_kc, in_=bkc_h[:, :])

        ebias = consts.tile([128, 2], F32, name="ebias")  # exp bias: [0]=0, [1]=ctx -2
        nc.vector.memset(ebias[:, 0:1], 0.0)
        nc.vector.memset(ebias[:, 1:2], -2.0)
        # ones row lives at partition 64 to match the PV-psum denominator row
        # (matmul fmap and weight must start at the same partition index)
        ones_sb = consts.tile([65, 64], BF16, name="ones_sb")
        nc.vector.memset(ones_sb[64:65, :], 1.0)
        dumm = consts.tile([128, 512], BF16, name="dumm")
        nc.vector.memset(dumm, 0.0)

        # ---- persistent activation tiles ----
        qT = [qkp.tile([128, T], BF16, name=f"qT{p}") for p in range(NPAIR)]
        kT = [qkp.tile([128, Tc + T], BF16, name=f"kT{p}") for p in range(NPAIR)]
        aT = [qkp.tile([128, T], BF16, name=f"aT{p}") for p in range(NPAIR)]
        obuf = qkp.tile([128, 8, T], BF16, name="obuf")
        v_sb = [vtp.tile([128, 8, 65], BF16, name=f"v{c}") for c in range(8)]
        for c in range(8):
            nc.vector.memset(v_sb[c][:, :, 64:65], 1.0)

        # ---- PE warm-up: junk matmuls so HAM un-throttles before real work ----
        warm_ps = scp.tile([128, 2, 512], F32, tag="mm", name="warm_ps")
        for i in range(NWARM):
            nc.tensor.matmul(
                warm_ps[:, i % 2, :], dumm[:, 0:128], dumm[:, :],
                start=True, stop=True, skip_group_check=True)

        # ---- input loads: x/wq/wk at top priority so qk(0) starts ASAP ----
        x_sb = wts.tile([128, 8, T], BF16, name="x_sb")
        xr = x_h[:, :].rearrange("(kc p) t -> p kc t", p=128)
        nc.sync.dma_start(out=x_sb[:, 0:4, :], in_=xr[:, 0:4, :])
        nc.scalar.dma_start(out=x_sb[:, 4:8, :], in_=xr[:, 4:8, :])
        wq_sb = wts.tile([128, 8, 512], BF16, name="wq_sb")
        nc.sync.dma_start(out=wq_sb, in_=wq_h[:, :].rearrange("(kc p) f -> p kc f", p=128))
        wk_sb = wts.tile([128, 8, 512], BF16, name="wk_sb")
        nc.scalar.dma_start(out=wk_sb, in_=wk_h[:, :].rearrange("(kc p) f -> p kc f", p=128))
        ctx_sb = wts.tile([128, 8, Tc], BF16, name="ctx_sb")
        nc.sync.dma_start(out=ctx_sb, in_=ctx_h[:, :].rearrange("(kc p) t -> p kc t", p=128))
        wkc_sb = wts.tile([128, 8, 512], BF16, name="wkc_sb")
        nc.sync.dma_start(out=wkc_sb, in_=wkc_h[:, :].rearrange("(kc p) f -> p kc f", p=128))
        wvc_sb = wts.tile([128, 8, 512], BF16, name="wvc_sb")
        nc.scalar.dma_start(out=wvc_sb, in_=wvc_h[:, :].rearrange("(kc p) f -> p kc f", p=128))

        bvb = consts.tile([128, 512], F32, name="bvb")
        nc.gpsimd.dma_start(out=bvb, in_=_bc0(bv_h[:, :], 128))
        bvcb = consts.tile([128, 512], F32, name="bvcb")
        nc.gpsimd.dma_start(out=bvcb, in_=_bc0(bvc_h[:, :], 128))
        wv_sb = wts.tile([128, 8, 512], BF16, name="wv_sb")
        nc.gpsimd.dma_start(out=wv_sb, in_=wv_h[:, :].rearrange("(kc p) f -> p kc f", p=128))
        wpj_sb = wts.tile([128, 4, 1024], BF16, name="wpj_sb")
        nc.gpsimd.dma_start(out=wpj_sb, in_=wpj_h[:, :].rearrange("(kc p) o -> p kc o", p=128))

        # ---- psum-group emitters (each allocates one "mm" tile) ----
        def emit_ctx():
            # kcT: context keys, feature-major, into kT[p][:, 0:Tc]
            for g in range(2):
                pt = scp.tile([128, 2, 512], F32, tag="mm", name=f"pkc{g}")
                for h in range(2):
                    f = 2 * g + h
                    for kc in range(8):
                        nc.tensor.matmul(
                            pt[:, h, 0:Tc], wkc_sb[:, kc, 128 * f:128 * f + 128],
                            ctx_sb[:, kc, :], start=(kc == 0), stop=(kc == 7))
                for h in range(2):
                    f = 2 * g + h
                    nc.vector.tensor_scalar_add(
                        out=kT[f][:, 0:Tc], in0=pt[:, h, 0:Tc],
                        scalar1=bias_kc[:, f:f + 1])

        def qk_filler(p, which, ti):
            w_sb = wq_sb if which == 0 else wk_sb
            dest = qT[p] if which == 0 else kT[p]
            dcol = 0 if which == 0 else Tc
            bcol = p if which == 0 else 4 + p
            ts, te = TCH[ti]

            def f():
                pt = scp.tile([128, 2, 512], F32, tag="mm", name=f"pqk{p}{which}{ti}")
                for kc in range(8):
                    nc.tensor.matmul(
                        pt[:, 0, 0:te - ts], w_sb[:, kc, 128 * p:128 * p + 128],
                        x_sb[:, kc, ts:te], start=(kc == 0), stop=(kc == 7))
                nc.vector.tensor_scalar_add(
                    out=dest[:, dcol + ts:dcol + te], in0=pt[:, 0, 0:te - ts],
                    scalar1=bias_qk[:, bcol:bcol + 1])
            return f

        def emit_vc():
            pt = scp.tile([128, 2, 512], F32, tag="mm", name="pvc")
            for kc in range(8):
                nc.tensor.matmul(
                    pt[:, 0, :], ctx_sb[:, kc, :], wvc_sb[:, kc, :],
                    start=(kc == 0), stop=(kc == 7))
            nc.vector.tensor_add(
                out=v_sb[0][:, :, 0:64],
                in0=pt[:, 0, :].rearrange("p (h d) -> p h d", h=8),
                in1=bvcb.rearrange("p (h d) -> p h d", h=8))

        def emit_v():
            # v natural layout [tok-chunk, head, 64] (+ ones col for denominator)
            for g in range(4):
                tts = [tt for tt in (2 * g, 2 * g + 1) if tt < 7]
                pt = scp.tile([128, 2, 512], F32, tag="mm", name=f"pv{g}")
                for h, tt in enumerate(tts):
                    for kc in range(8):
                        nc.tensor.matmul(
                            pt[:, h, :], x_sb[:, kc, 128 * tt:128 * tt + 128],
                            wv_sb[:, kc, :], start=(kc == 0), stop=(kc == 7))
                for h, tt in enumerate(tts):
                    nc.vector.tensor_add(
                        out=v_sb[1 + tt][:, :, 0:64],
                        in0=pt[:, h, :].rearrange("p (h d) -> p h d", h=8),
                        in1=bvb.rearrange("p (h d) -> p h d", h=8))

        def out_filler(of, ti):
            ts, te = TCH[ti]

            def f():
                pt = scp.tile([128, 2, 512], F32, tag="mm", name=f"po{of}{ti}")
                for kc in range(4):
                    nc.tensor.matmul(
                        pt[:, 0, 0:te - ts], wpj_sb[:, kc, 128 * of:128 * of + 128],
                        aT[kc][:, ts:te], start=(kc == 0), stop=(kc == 3))
                nc.scalar.copy(out=obuf[:, of, ts:te], in_=pt[:, 0, 0:te - ts])
                if ti == 1:
                    nc.sync.dma_start(
                        out=out_h[128 * of:128 * of + 128, :], in_=obuf[:, of, :])
            return f

        fillers = []

        def emit_att(p, ti):
            ts, te = TCH[ti]
            w = te - ts
            lives = [c for c in range(8) if max(128 * (c - 1), ts) < te]
            n = len(lives)
            at = pvp.tile([65, 2, 512], F32, tag="pv", name=f"at{p}{ti}")
            pbs = {}

            def S(c):
                cs = max(128 * (c - 1), ts)
                sc = scp.tile([128, 2, 512], F32, tag="mm", name=f"sc{p}{ti}{c}")
                for hi in range(2):
                    nc.tensor.matmul(
                        sc[:, hi, cs - ts:w],
                        kT[p][64 * hi:64 * hi + 64, 128 * c:128 * c + 128],
                        qT[p][64 * hi:64 * hi + 64, cs:te],
                        start=True, stop=True, tile_position=(64 * hi, 0))
                pb = probsp.tile([128, 2, 512], BF16, tag="pb", name=f"pb{p}{ti}{c}")
                nc.scalar.activation(
                    out=pb[:, :, cs - ts:w], in_=sc[:, :, cs - ts:w], func=AF.Exp,
                    bias=(ebias[:, 1:2] if c == 0 else ebias[:, 0:1]), scale=0.125)
                for hi in range(2):
                    mask_eng = nc.vector if hi == 0 else nc.gpsimd
                    for s0, e0, mc in _band_pieces(c, ts, te):
                        mask_eng.tensor_mul(
                            out=pb[:, hi, s0 - ts:e0 - ts],
                            in0=pb[:, hi, s0 - ts:e0 - ts],
                            in1=maskband[:, mc:mc + (e0 - s0)])
                pbs[c] = (pb, cs)

            def P(c):
                pb, cs = pbs.pop(c)
                for hi in range(2):
                    nc.tensor.matmul(
                        at[0:65, hi, cs - ts:w], v_sb[c][:, 2 * p + hi, :],
                        pb[:, hi, cs - ts:w],
                        start=(c == lives[0]), stop=(c == lives[-1]),
                        skip_group_check=True)

            si = 0

            def pushS():
                nonlocal si
                if si < n:
                    S(lives[si])
                    si += 1

            pushS()
            pushS()
            for pi in range(n):
                if fillers and pi >= 1:
                    fillers.pop(0)()
                elif si - pi < 4:
                    pushS()  # no filler: deepen score-ahead instead (cap 3)
                pushS()
                P(lives[pi])

            # evacuate PV psum: cast to bf16 staging (DVE), shift to aT via DMA
            stg = smallp.tile([65, 2, 512], BF16, tag="stg", name=f"stg{p}{ti}")
            nc.scalar.copy(out=stg[:, 0, 0:w], in_=at[:, 0, 0:w])
            nc.vector.tensor_copy(out=stg[:, 1, 0:w], in_=at[:, 1, 0:w])
            nc.sync.dma_start(out=aT[p][0:64, ts:te], in_=stg[0:64, 0, 0:w])
            nc.sync.dma_start(out=aT[p][64:128, ts:te], in_=stg[0:64, 1, 0:w])

            def finish():
                # broadcast raw denom row across partitions (K=1 ones-matmul),
                # then 128-lane-parallel fast reciprocal, then normalize aT
                rb = scp.tile([128, 2, 512], F32, tag="mm", name=f"rb{p}{ti}")
                nc.tensor.matmul(
                    rb[0:64, 0, 0:w], ones_sb[64:65, :], stg[64:65, 0, 0:w],
                    start=True, stop=True, tile_position=(64, 0),
                    skip_group_check=True)
                nc.tensor.matmul(
                    rb[64:128, 0, 0:w], ones_sb[64:65, :], stg[64:65, 1, 0:w],
                    start=True, stop=True, tile_position=(64, 64),
                    skip_group_check=True)
                rbc = smallp.tile([128, 512], F32, tag="rbc", name=f"rbc{p}{ti}")
                nc.vector.reciprocal_approx_fast(out=rbc[:, 0:w], in_=rb[:, 0, 0:w])
                nc.vector.tensor_mul(
                    out=aT[p][:, ts:te], in0=aT[p][:, ts:te], in1=rbc[:, 0:w])

            fillers.insert(0, finish)

        # ---- master schedule ----
        emit_ctx()
        for which in range(2):
            for ti in range(2):
                qk_filler(0, which, ti)()
        emit_vc()
        emit_v()
        for p in range(1, NPAIR):
            for which in range(2):
                for ti in range(2):
                    fillers.append(qk_filler(p, which, ti))

        emit_att(0, 0)
        emit_att(0, 1)
        emit_att(1, 0)
        emit_att(1, 1)
        emit_att(2, 0)
        emit_att(2, 1)
        emit_att(3, 0)
        # att(3,0)'s finish (normalizes aT[3][:, t0]) must precede out-t0 reads
        fillers.extend(out_filler(of, 0) for of in range(8))
        emit_att(3, 1)
        fillers.extend(out_filler(of, 1) for of in range(8))
        while fillers:
            fillers.pop(0)()

    if not nc.is_finalized():
        nc.finalize()
    return nc


_NC_CACHE = {}


def _get_nc():
    if "nc" not in _NC_CACHE:
        _NC_CACHE["nc"] = build_nc()
    return _NC_CACHE["nc"]


def _pack128(v):
    """[128*n] -> [128, n] with [p, f] = v[128*f + p]."""
    n = v.shape[0] // 128
    return np.ascontiguousarray(v.reshape(n, 128).T)


def make_in_maps(inputs):
    bf16 = ml_dtypes.bfloat16
    x = np.asarray(inputs["x"], np.float32)
    ctx_seq = np.asarray(inputs["context_seq"], np.float32)
    w_ref = np.asarray(inputs["w_ref"], np.float32)
    b_ref = np.asarray(inputs["b_ref"], np.float32)
    w_attn = np.asarray(inputs["w_attn"], np.float32)
    b_attn = np.asarray(inputs["b_attn"], np.float32)
    w_proj = np.asarray(inputs["w_proj"], np.float32)

    # mask band constant: cols 0-127 causal (1 where q>=p), cols 128-255
    # anti-diagonal (0 where q==p else 1)
    qq = np.arange(128)[None, :]
    pp = np.arange(128)[:, None]
    mband = np.concatenate([(qq >= pp), (qq != pp)], axis=1).astype(bf16)
    mband = np.ascontiguousarray(mband)

    in_maps = []
    for b in range(4):
        xT = np.ascontiguousarray(x[b].T.astype(bf16))
        ctxT = np.ascontiguousarray(ctx_seq[b].T.astype(bf16))
        for g in range(2):
            sl = slice(512 * g, 512 * g + 512)
            in_maps.append(dict(
                xT=xT,
                ctxT=ctxT,
                w_q=np.ascontiguousarray(w_attn[:, 0 * NX:1 * NX][:, sl].astype(bf16)),
                w_k=np.ascontiguousarray(w_attn[:, 1 * NX:2 * NX][:, sl].astype(bf16)),
                w_v=np.ascontiguousarray(w_attn[:, 2 * NX:3 * NX][:, sl].astype(bf16)),
                w_kc=np.ascontiguousarray(w_ref[:, 0 * NX:1 * NX][:, sl].astype(bf16)),
                w_vc=np.ascontiguousarray(w_ref[:, 1 * NX:2 * NX][:, sl].astype(bf16)),
                w_pj=np.ascontiguousarray(w_proj[sl, :].astype(bf16)),
                b_qk=_pack128(np.concatenate([b_attn[0 * NX:1 * NX][sl],
                                              b_attn[1 * NX:2 * NX][sl]])),
                b_kc=_pack128(b_ref[0 * NX:1 * NX][sl]),
                b_v=np.ascontiguousarray(b_attn[2 * NX:3 * NX][sl].reshape(1, 512)),
                b_vc=np.ascontiguousarray(b_ref[1 * NX:2 * NX][sl].reshape(1, 512)),
                mband=mband,
            ))
    return in_maps


def kernel(**inputs):
    b_proj = np.asarray(inputs["b_proj"], np.float32)
    in_maps = make_in_maps(inputs)
    nc = _get_nc()
    res = run_bass_kernel_spmd(nc, in_maps, core_ids=list(range(8)),
                               trace=os.environ.get("COCON_TRACE", "") == "1")
    outs = res.results
    out = np.empty((4, T, NX), np.float32)
    for b in range(4):
        acc = outs[2 * b]["outT"].astype(np.float32) + \
            outs[2 * b + 1]["outT"].astype(np.float32)  # [1024, 896]
        out[b] = acc.T + b_proj[None, :]
    if res.exec_time_ns is not None:
        kernel.last_exec_time_ns = res.exec_time_ns
    return out


kernel.last_exec_time_ns = None


# revision 11
# speedup vs baseline: 1.4864x; 1.1333x over previous
"""Trainium2 Bass kernel for nn_CoconAttention (dense transformer attention block).

Sharding: 8 cores = 4 batches x 2 head-groups (8 heads each). Each core gets
pre-transposed/sliced bf16 inputs, computes its partial output outT [1024, 896]
(bf16, transposed, pre-b_proj), and the host sums head-group pairs + transposes.

v3 design (all-bf16 matmul path, gap-filled PE schedule):
  - every matmul operand bf16 (FWL weight loads, half the HBM traffic)
  - loads spread across the three DMA queues (sync/scalar HW-DGE + gpsimd)
  - PE warm-up stream so HAM un-throttles before real work
  - attention software-pipelined: scores run 2-3 chunks ahead of PV, and
    projection / out-proj / normalize psum-groups are popped from a filler
    queue into the gaps so the in-order PE never waits on the exp/mask chain
  - PV psum leaves via DVE cast-copy then a plain DMA partition-shift; the
    denominator row is reciprocal'd (fast approx) and broadcast across
    partitions with a K=1 ones-matmul (no DRAM bounce)
  - masks: hi0 on DVE, hi1 on Pool (parallel engines)
"""
import os
import sys

import numpy as np
import ml_dtypes

try:
    import concourse.bass as bass
except ImportError:  # fresh grading dir: fall back to the repo location
    sys.path.insert(0, "/opt/trn_rl_repo")
    import concourse.bass as bass
import concourse.bacc as bacc

import concourse.tile as tile
from concourse import mybir
from concourse.bass_utils import run_bass_kernel_spmd
from contextlib import ExitStack

F32 = mybir.dt.float32
BF16 = mybir.dt.bfloat16
AF = mybir.ActivationFunctionType

T, Tc, NX = 896, 128, 1024
TCH = ((0, 512), (512, 896))  # tok chunks
NPAIR = 4  # head pairs per core
NWARM = 26  # PE warm-up matmuls


def _bc0(ap, n):
    """Partition-broadcast read AP: [1, ...] -> [n, ...] with partition step 0."""
    return bass.AP(tensor=ap.tensor, offset=ap.offset, ap=[[0, n]] + list(ap.ap[1:]))


def _band_pieces(c, ts, te):
    """Mask applications for chunk c in [ts,te): (s0, e0, mask_col_offset)."""
    if c == 0:
        bs, be, moff, borig = 0, 128, 128, 0  # diag half only
    elif c <= 6:
        bs = 128 * (c - 1)
        be, moff, borig = bs + 256, 0, bs  # causal(128) + diag(128)
    else:
        bs, be, moff, borig = 768, 896, 0, 768  # causal half only
    s0, e0 = max(bs, ts), min(be, te)
    if s0 >= e0:
        return []
    return [(s0, e0, moff + (s0 - borig))]


def build_nc():
    nc = bacc.Bacc("TRN2", target_bir_lowering=False)

    # all big inputs staged partition-major on host: [128, kc, cols] so each
    # partition's DMA row is one large contiguous run (fast descriptors)
    x_h = nc.dram_tensor("xT", [128, 8, T], BF16, kind="ExternalInput")
    ctx_h = nc.dram_tensor("ctxT", [128, 8, Tc], BF16, kind="ExternalInput")
    wq_h = nc.dram_tensor("w_q", [128, 8, 512], BF16, kind="ExternalInput")
    wk_h = nc.dram_tensor("w_k", [128, 8, 512], BF16, kind="ExternalInput")
    wv_h = nc.dram_tensor("w_v", [128, 8, 512], BF16, kind="ExternalInput")
    wkc_h = nc.dram_tensor("w_kc", [128, 8, 512], BF16, kind="ExternalInput")
    wvc_h = nc.dram_tensor("w_vc", [128, 8, 512], BF16, kind="ExternalInput")
    wpj_h = nc.dram_tensor("w_pj", [128, 4, NX], BF16, kind="ExternalInput")
    bqk_h = nc.dram_tensor("b_qk", [128, 8], F32, kind="ExternalInput")
    bkc_h = nc.dram_tensor("b_kc", [128, 4], F32, kind="ExternalInput")
    bv_h = nc.dram_tensor("b_v", [1, 512], F32, kind="ExternalInput")
    bvc_h = nc.dram_tensor("b_vc", [1, 512], F32, kind="ExternalInput")
    mb_h = nc.dram_tensor("mband", [128, 256], BF16, kind="ExternalInput")
    out_h = nc.dram_tensor("outT", [NX, T], BF16, kind="ExternalOutput")

    with tile.TileContext(nc) as tc, ExitStack() as top:
        consts = top.enter_context(tc.tile_pool(name="consts", bufs=1))
        wts = top.enter_context(tc.tile_pool(name="wts", bufs=1))
        qkp = top.enter_context(tc.tile_pool(name="qkp", bufs=1))
        vtp = top.enter_context(tc.tile_pool(name="vtp", bufs=1))
        probsp = top.enter_context(tc.tile_pool(name="probsp", bufs=4))
        smallp = top.enter_context(tc.tile_pool(name="smallp", bufs=2))
        scp = top.enter_context(tc.tile_pool(name="scp", bufs=3, space="PSUM"))
        pvp = top.enter_context(tc.tile_pool(name="pvp", bufs=1, space="PSUM"))

        # ---- constants (sync queue, tiny) ----
        maskband = consts.tile([128, 256], BF16, name="maskband")
        nc.gpsimd.dma_start(out=maskband, in_=mb_h[:, :])
        bias_qk = consts.tile([128, 8], F32, name="bias_qk")
        nc.sync.dma_start(out=bias_qk, in_=bqk_h[:, :])
        bias_kc = consts.tile([128, 4], F32, name="bias_kc")
        nc.sync.dma_start(out=bias_kc, in_=bkc_h[:, :])

        ebias = consts.tile([128, 2], F32, name="ebias")  # exp bias: [0]=0, [1]=ctx -2
        nc.vector.memset(ebias[:, 0:1], 0.0)
        nc.vector.memset(ebias[:, 1:2], -2.0)
        # ones row lives at partition 64 to match the PV-psum denominator row
        # (matmul fmap and weight must start at the same partition index)
        ones_sb = consts.tile([65, 64], BF16, name="ones_sb")
        nc.vector.memset(ones_sb[64:65, :], 1.0)
        dumm = consts.tile([128, 512], BF16, name="dumm")
        nc.vector.memset(dumm, 0.0)

        # ---- persistent activation tiles ----
        qT = [qkp.tile([128, T], BF16, name=f"qT{p}") for p in range(NPAIR)]
        kT = [qkp.tile([128, Tc + T], BF16, name=f"kT{p}") for p in range(NPAIR)]
        aT = [qkp.tile([128, T], BF16, name=f"aT{p}") for p in range(NPAIR)]
        obuf = qkp.tile([128, 8, T], BF16, name="obuf")
        v_sb = [vtp.tile([128, 8, 65], BF16, name=f"v{c}") for c in range(8)]
        for c in range(8):
            nc.vector.memset(v_sb[c][:, :, 64:65], 1.0)

        # ---- PE warm-up: junk matmuls so HAM un-throttles before real work ----
        warm_ps = scp.tile([128, 2, 512], F32, tag="mm", name="warm_ps")
        for i in range(NWARM):
            nc.tensor.matmul(
                warm_ps[:, i % 2, :], dumm[:, 0:128], dumm[:, :],
                start=True, stop=True, skip_group_check=True)

        # ---- input loads: contiguous partition-major, critical-first ----
        x_sb = wts.tile([128, 8, T], BF16, name="x_sb")
        nc.sync.dma_start(out=x_sb, in_=x_h[:, :, :])
        wq_sb = wts.tile([128, 8, 512], BF16, name="wq_sb")
        nc.scalar.dma_start(out=wq_sb, in_=wq_h[:, :, :])
        wk_sb = wts.tile([128, 8, 512], BF16, name="wk_sb")
        nc.scalar.dma_start(out=wk_sb, in_=wk_h[:, :, :])
        ctx_sb = wts.tile([128, 8, Tc], BF16, name="ctx_sb")
        nc.gpsimd.dma_start(out=ctx_sb, in_=ctx_h[:, :, :])
        wkc_sb = wts.tile([128, 8, 512], BF16, name="wkc_sb")
        nc.gpsimd.dma_start(out=wkc_sb, in_=wkc_h[:, :, :])
        wvc_sb = wts.tile([128, 8, 512], BF16, name="wvc_sb")
        nc.gpsimd.dma_start(out=wvc_sb, in_=wvc_h[:, :, :])
        wv_sb = wts.tile([128, 8, 512], BF16, name="wv_sb")
        nc.gpsimd.dma_start(out=wv_sb, in_=wv_h[:, :, :])
        bvb = consts.tile([128, 512], F32, name="bvb")
        nc.gpsimd.dma_start(out=bvb, in_=_bc0(bv_h[:, :], 128))
        bvcb = consts.tile([128, 512], F32, name="bvcb")
        nc.gpsimd.dma_start(out=bvcb, in_=_bc0(bvc_h[:, :], 128))
        wpj_sb = wts.tile([128, 4, 1024], BF16, name="wpj_sb")
        nc.sync.dma_start(out=wpj_sb, in_=wpj_h[:, :, :])

        # ---- psum-group emitters (each allocates one "mm" tile) ----
        def emit_ctx():
            # kcT: context keys, feature-major, into kT[p][:, 0:Tc]
            for g in range(2):
                pt = scp.tile([128, 2, 512], F32, tag="mm", name=f"pkc{g}")
                for h in range(2):
                    f = 2 * g + h
                    for kc in range(8):
                        nc.tensor.matmul(
                            pt[:, h, 0:Tc], wkc_sb[:, kc, 128 * f:128 * f + 128],
                            ctx_sb[:, kc, :], start=(kc == 0), stop=(kc == 7))
                for h in range(2):
                    f = 2 * g + h
                    nc.vector.tensor_scalar_add(
                        out=kT[f][:, 0:Tc], in0=pt[:, h, 0:Tc],
                        scalar1=bias_kc[:, f:f + 1])

        def qk_filler(p, which, ti):
            w_sb = wq_sb if which == 0 else wk_sb
            dest = qT[p] if which == 0 else kT[p]
            dcol = 0 if which == 0 else Tc
            bcol = p if which == 0 else 4 + p
            ts, te = TCH[ti]

            def f():
                pt = scp.tile([128, 2, 512], F32, tag="mm", name=f"pqk{p}{which}{ti}")
                for kc in range(8):
                    nc.tensor.matmul(
                        pt[:, 0, 0:te - ts], w_sb[:, kc, 128 * p:128 * p + 128],
                        x_sb[:, kc, ts:te], start=(kc == 0), stop=(kc == 7))
                nc.vector.tensor_scalar_add(
                    out=dest[:, dcol + ts:dcol + te], in0=pt[:, 0, 0:te - ts],
                    scalar1=bias_qk[:, bcol:bcol + 1])
            return f

        def emit_vc():
            pt = scp.tile([128, 2, 512], F32, tag="mm", name="pvc")
            for kc in range(8):
                nc.tensor.matmul(
                    pt[:, 0, :], ctx_sb[:, kc, :], wvc_sb[:, kc, :],
                    start=(kc == 0), stop=(kc == 7))
            nc.vector.tensor_add(
                out=v_sb[0][:, :, 0:64],
                in0=pt[:, 0, :].rearrange("p (h d) -> p h d", h=8),
                in1=bvcb.rearrange("p (h d) -> p h d", h=8))

        def emit_v():
            # v natural layout [tok-chunk, head, 64] (+ ones col for denominator)
            for g in range(4):
                tts = [tt for tt in (2 * g, 2 * g + 1) if tt < 7]
                pt = scp.tile([128, 2, 512], F32, tag="mm", name=f"pv{g}")
                for h, tt in enumerate(tts):
                    for kc in range(8):
                        nc.tensor.matmul(
                            pt[:, h, :], x_sb[:, kc, 128 * tt:128 * tt + 128],
                            wv_sb[:, kc, :], start=(kc == 0), stop=(kc == 7))
                for h, tt in enumerate(tts):
                    nc.vector.tensor_add(
                        out=v_sb[1 + tt][:, :, 0:64],
                        in0=pt[:, h, :].rearrange("p (h d) -> p h d", h=8),
                        in1=bvb.rearrange("p (h d) -> p h d", h=8))

        def out_filler(of, ti):
            ts, te = TCH[ti]

            def f():
                pt = scp.tile([128, 2, 512], F32, tag="mm", name=f"po{of}{ti}")
                for kc in range(4):
                    nc.tensor.matmul(
                        pt[:, 0, 0:te - ts], wpj_sb[:, kc, 128 * of:128 * of + 128],
                        aT[kc][:, ts:te], start=(kc == 0), stop=(kc == 3))
                nc.scalar.copy(out=obuf[:, of, ts:te], in_=pt[:, 0, 0:te - ts])
                if ti == 1:
                    nc.sync.dma_start(
                        out=out_h[128 * of:128 * of + 128, :], in_=obuf[:, of, :])
            return f

        fillers = []

        def emit_att(p, ti):
            ts, te = TCH[ti]
            w = te - ts
            lives = [c for c in range(8) if max(128 * (c - 1), ts) < te]
            n = len(lives)
            at = pvp.tile([65, 2, 512], F32, tag="pv", name=f"at{p}{ti}")
            pbs = {}

            def S(c):
                cs = max(128 * (c - 1), ts)
                sc = scp.tile([128, 2, 512], F32, tag="mm", name=f"sc{p}{ti}{c}")
                for hi in range(2):
                    nc.tensor.matmul(
                        sc[:, hi, cs - ts:w],
                        kT[p][64 * hi:64 * hi + 64, 128 * c:128 * c + 128],
                        qT[p][64 * hi:64 * hi + 64, cs:te],
                        start=True, stop=True, tile_position=(64 * hi, 0))
                pb = probsp.tile([128, 2, 512], BF16, tag="pb", name=f"pb{p}{ti}{c}")
                nc.scalar.activation(
                    out=pb[:, :, cs - ts:w], in_=sc[:, :, cs - ts:w], func=AF.Exp,
                    bias=(ebias[:, 1:2] if c == 0 else ebias[:, 0:1]), scale=0.125)
                for hi in range(2):
                    mask_eng = nc.vector if hi == 0 else nc.gpsimd
                    for s0, e0, mc in _band_pieces(c, ts, te):
                        mask_eng.tensor_mul(
                            out=pb[:, hi, s0 - ts:e0 - ts],
                            in0=pb[:, hi, s0 - ts:e0 - ts],
                            in1=maskband[:, mc:mc + (e0 - s0)])
                pbs[c] = (pb, cs)

            def P(c):
                pb, cs = pbs.pop(c)
                for hi in range(2):
                    nc.tensor.matmul(
                        at[0:65, hi, cs - ts:w], v_sb[c][:, 2 * p + hi, :],
                        pb[:, hi, cs - ts:w],
                        start=(c == lives[0]), stop=(c == lives[-1]),
                        skip_group_check=True)

            si = 0

            def pushS():
                nonlocal si
                if si < n:
                    S(lives[si])
                    si += 1

            pushS()
            pushS()
            for pi in range(n):
                if fillers and pi >= 2:
                    fillers.pop(0)()
                elif si - pi < 4:
                    pushS()  # no filler: deepen score-ahead instead (cap 3)
                pushS()
                P(lives[pi])

            # evacuate PV psum: cast to bf16 staging (DVE), shift to aT via DMA
            stg = smallp.tile([65, 2, 512], BF16, tag="stg", name=f"stg{p}{ti}")
            nc.scalar.copy(out=stg[:, 0, 0:w], in_=at[:, 0, 0:w])
            nc.vector.tensor_copy(out=stg[:, 1, 0:w], in_=at[:, 1, 0:w])
            nc.sync.dma_start(out=aT[p][0:64, ts:te], in_=stg[0:64, 0, 0:w])
            nc.sync.dma_start(out=aT[p][64:128, ts:te], in_=stg[0:64, 1, 0:w])

            def finish():
                # broadcast raw denom row across partitions (K=1 ones-matmul),
                # then 128-lane-parallel fast reciprocal, then normalize aT
                rb = scp.tile([128, 2, 512], F32, tag="mm", name=f"rb{p}{ti}")
                nc.tensor.matmul(
                    rb[0:64, 0, 0:w], ones_sb[64:65, :], stg[64:65, 0, 0:w],
                    start=True, stop=True, tile_position=(64, 0),
                    skip_group_check=True)
                nc.tensor.matmul(
                    rb[64:128, 0, 0:w], ones_sb[64:65, :], stg[64:65, 1, 0:w],
                    start=True, stop=True, tile_position=(64, 64),
                    skip_group_check=True)
                rbc = smallp.tile([128, 512], F32, tag="rbc", name=f"rbc{p}{ti}")
                nc.vector.reciprocal_approx_fast(out=rbc[:, 0:w], in_=rb[:, 0, 0:w])
                nc.vector.tensor_mul(
                    out=aT[p][:, ts:te], in0=aT[p][:, ts:te], in1=rbc[:, 0:w])

            fillers.insert(0, finish)

        # ---- master schedule ----
        emit_ctx()
        for which in range(2):
            for ti in range(2):
                qk_filler(0, which, ti)()
        emit_vc()
        emit_v()
        for p in range(1, NPAIR):
            for which in range(2):
                for ti in range(2):
                    fillers.append(qk_filler(p, which, ti))

        emit_att(0, 0)
        emit_att(0, 1)
        emit_att(1, 0)
        emit_att(1, 1)
        emit_att(2, 0)
        emit_att(2, 1)
        emit_att(3, 0)
        # att(3,0)'s finish (normalizes aT[3][:, t0]) must precede out-t0 reads
        fillers.extend(out_filler(of, 0) for of in range(8))
        emit_att(3, 1)
        fillers.extend(out_filler(of, 1) for of in range(8))
        while fillers:
            fillers.pop(0)()

    if not nc.is_finalized():
        nc.finalize()
    return nc


_NC_CACHE = {}


def _get_nc():
    if "nc" not in _NC_CACHE:
        _NC_CACHE["nc"] = build_nc()
    return _NC_CACHE["nc"]


def _pack128(v):
    """[128*n] -> [128, n] with [p, f] = v[128*f + p]."""
    n = v.shape[0] // 128
    return np.ascontiguousarray(v.reshape(n, 128).T)


def make_in_maps(inputs):
    bf16 = ml_dtypes.bfloat16
    x = np.asarray(inputs["x"], np.float32)
    ctx_seq = np.asarray(inputs["context_seq"], np.float32)
    w_ref = np.asarray(inputs["w_ref"], np.float32)
    b_ref = np.asarray(inputs["b_ref"], np.float32)
    w_attn = np.asarray(inputs["w_attn"], np.float32)
    b_attn = np.asarray(inputs["b_attn"], np.float32)
    w_proj = np.asarray(inputs["w_proj"], np.float32)

    # mask band constant: cols 0-127 causal (1 where q>=p), cols 128-255
    # anti-diagonal (0 where q==p else 1)
    qq = np.arange(128)[None, :]
    pp = np.arange(128)[:, None]
    mband = np.concatenate([(qq >= pp), (qq != pp)], axis=1).astype(bf16)
    mband = np.ascontiguousarray(mband)

    def _pm(a, nkc=8):
        # [nkc*128, cols] -> partition-major [128, nkc, cols]
        return np.ascontiguousarray(
            a.reshape(nkc, 128, a.shape[1]).transpose(1, 0, 2))

    in_maps = []
    for b in range(4):
        xT = _pm(x[b].T.astype(bf16))
        ctxT = _pm(ctx_seq[b].T.astype(bf16))
        for g in range(2):
            sl = slice(512 * g, 512 * g + 512)
            in_maps.append(dict(
                xT=xT,
                ctxT=ctxT,
                w_q=_pm(w_attn[:, 0 * NX:1 * NX][:, sl].astype(bf16)),
                w_k=_pm(w_attn[:, 1 * NX:2 * NX][:, sl].astype(bf16)),
                w_v=_pm(w_attn[:, 2 * NX:3 * NX][:, sl].astype(bf16)),
                w_kc=_pm(w_ref[:, 0 * NX:1 * NX][:, sl].astype(bf16)),
                w_vc=_pm(w_ref[:, 1 * NX:2 * NX][:, sl].astype(bf16)),
                w_pj=_pm(w_proj[sl, :].astype(bf16), nkc=4),
                b_qk=_pack128(np.concatenate([b_attn[0 * NX:1 * NX][sl],
                                              b_attn[1 * NX:2 * NX][sl]])),
                b_kc=_pack128(b_ref[0 * NX:1 * NX][sl]),
                b_v=np.ascontiguousarray(b_attn[2 * NX:3 * NX][sl].reshape(1, 512)),
                b_vc=np.ascontiguousarray(b_ref[1 * NX:2 * NX][sl].reshape(1, 512)),
                mband=mband,
            ))
    return in_maps


def kernel(**inputs):
    b_proj = np.asarray(inputs["b_proj"], np.float32)
    in_maps = make_in_maps(inputs)
    nc = _get_nc()
    res = run_bass_kernel_spmd(nc, in_maps, core_ids=list(range(8)),
                               trace=os.environ.get("COCON_TRACE", "") == "1")
    outs = res.results
    out = np.empty((4, T, NX), np.float32)
    for b in range(4):
        acc = outs[2 * b]["outT"].astype(np.float32) + \
            outs[2 * b + 1]["outT"].astype(np.float32)  # [1024, 896]
        out[b] = acc.T + b_proj[None, :]
    if res.exec_time_ns is not None:
        kernel.last_exec_time_ns = res.exec_time_ns
    return out


kernel.last_exec_time_ns = None


# revision 12
# speedup vs baseline: 1.5284x; 1.0283x over previous
"""Trainium2 Bass kernel for nn_CoconAttention (dense transformer attention block).

Sharding: 8 cores = 4 batches x 2 head-groups (8 heads each). Each core gets
pre-transposed/sliced bf16 inputs, computes its partial output outT [1024, 896]
(bf16, transposed, pre-b_proj), and the host sums head-group pairs + transposes.

v3 design (all-bf16 matmul path, gap-filled PE schedule):
  - every matmul operand bf16 (FWL weight loads, half the HBM traffic)
  - loads spread across the three DMA queues (sync/scalar HW-DGE + gpsimd)
  - PE warm-up stream so HAM un-throttles before real work
  - attention software-pipelined: scores run 2-3 chunks ahead of PV, and
    projection / out-proj / normalize psum-groups are popped from a filler
    queue into the gaps so the in-order PE never waits on the exp/mask chain
  - PV psum leaves via DVE cast-copy then a plain DMA partition-shift; the
    denominator row is reciprocal'd (fast approx) and broadcast across
    partitions with a K=1 ones-matmul (no DRAM bounce)
  - masks: hi0 on DVE, hi1 on Pool (parallel engines)
"""
import os
import sys

import numpy as np
import ml_dtypes

try:
    import concourse.bass as bass
except ImportError:  # fresh grading dir: fall back to the repo location
    sys.path.insert(0, "/opt/trn_rl_repo")
    import concourse.bass as bass
import concourse.bacc as bacc

import concourse.tile as tile
from concourse import mybir
from concourse.bass_utils import run_bass_kernel_spmd
from contextlib import ExitStack

F32 = mybir.dt.float32
BF16 = mybir.dt.bfloat16
AF = mybir.ActivationFunctionType

T, Tc, NX = 896, 128, 1024
TCH = ((0, 512), (512, 896))  # tok chunks
NPAIR = 4  # head pairs per core
NWARM = 28  # PE warm-up matmuls


def _bc0(ap, n):
    """Partition-broadcast read AP: [1, ...] -> [n, ...] with partition step 0."""
    return bass.AP(tensor=ap.tensor, offset=ap.offset, ap=[[0, n]] + list(ap.ap[1:]))


def _band_pieces(c, ts, te):
    """Mask applications for chunk c in [ts,te): (s0, e0, mask_col_offset)."""
    if c == 0:
        bs, be, moff, borig = 0, 128, 128, 0  # diag half only
    elif c <= 6:
        bs = 128 * (c - 1)
        be, moff, borig = bs + 256, 0, bs  # causal(128) + diag(128)
    else:
        bs, be, moff, borig = 768, 896, 0, 768  # causal half only
    s0, e0 = max(bs, ts), min(be, te)
    if s0 >= e0:
        return []
    return [(s0, e0, moff + (s0 - borig))]


def build_nc():
    nc = bacc.Bacc("TRN2", target_bir_lowering=False)

    # all big inputs staged partition-major on host: [128, kc, cols] so each
    # partition's DMA row is one large contiguous run (fast descriptors)
    x_h = nc.dram_tensor("xT", [128, 8, T], BF16, kind="ExternalInput")
    ctx_h = nc.dram_tensor("ctxT", [128, 8, Tc], BF16, kind="ExternalInput")
    wq_h = nc.dram_tensor("w_q", [128, 8, 512], BF16, kind="ExternalInput")
    wk_h = nc.dram_tensor("w_k", [128, 8, 512], BF16, kind="ExternalInput")
    wv_h = nc.dram_tensor("w_v", [128, 8, 512], BF16, kind="ExternalInput")
    wkc_h = nc.dram_tensor("w_kc", [128, 8, 512], BF16, kind="ExternalInput")
    wvc_h = nc.dram_tensor("w_vc", [128, 8, 512], BF16, kind="ExternalInput")
    wpj_h = nc.dram_tensor("w_pj", [128, 4, NX], BF16, kind="ExternalInput")
    bqk_h = nc.dram_tensor("b_qk", [128, 8], F32, kind="ExternalInput")
    bkc_h = nc.dram_tensor("b_kc", [128, 4], F32, kind="ExternalInput")
    bv_h = nc.dram_tensor("b_v", [1, 512], F32, kind="ExternalInput")
    bvc_h = nc.dram_tensor("b_vc", [1, 512], F32, kind="ExternalInput")
    mb_h = nc.dram_tensor("mband", [128, 256], BF16, kind="ExternalInput")
    out_h = nc.dram_tensor("outT", [NX, T], BF16, kind="ExternalOutput")

    with tile.TileContext(nc) as tc, ExitStack() as top:
        consts = top.enter_context(tc.tile_pool(name="consts", bufs=1))
        wts = top.enter_context(tc.tile_pool(name="wts", bufs=1))
        qkp = top.enter_context(tc.tile_pool(name="qkp", bufs=1))
        vtp = top.enter_context(tc.tile_pool(name="vtp", bufs=1))
        probsp = top.enter_context(tc.tile_pool(name="probsp", bufs=5))
        smallp = top.enter_context(tc.tile_pool(name="smallp", bufs=2))
        scp = top.enter_context(tc.tile_pool(name="scp", bufs=3, space="PSUM"))
        pvp = top.enter_context(tc.tile_pool(name="pvp", bufs=1, space="PSUM"))

        # ---- constants (sync queue, tiny) ----
        maskband = consts.tile([128, 256], BF16, name="maskband")
        nc.gpsimd.dma_start(out=maskband, in_=mb_h[:, :])
        bias_qk = consts.tile([128, 8], F32, name="bias_qk")
        nc.sync.dma_start(out=bias_qk, in_=bqk_h[:, :])
        bias_kc = consts.tile([128, 4], F32, name="bias_kc")
        nc.sync.dma_start(out=bias_kc, in_=bkc_h[:, :])

        ebias = consts.tile([128, 2], F32, name="ebias")  # exp bias: [0]=0, [1]=ctx -2
        nc.vector.memset(ebias[:, 0:1], 0.0)
        nc.vector.memset(ebias[:, 1:2], -2.0)
        # ones row lives at partition 64 to match the PV-psum denominator row
        # (matmul fmap and weight must start at the same partition index)
        ones_sb = consts.tile([65, 64], BF16, name="ones_sb")
        nc.vector.memset(ones_sb[64:65, :], 1.0)
        dumm = consts.tile([128, 512], BF16, name="dumm")
        nc.vector.memset(dumm, 0.0)

        # ---- persistent activation tiles ----
        qT = [qkp.tile([128, T], BF16, name=f"qT{p}") for p in range(NPAIR)]
        kT = [qkp.tile([128, Tc + T], BF16, name=f"kT{p}") for p in range(NPAIR)]
        aT = [qkp.tile([128, T], BF16, name=f"aT{p}") for p in range(NPAIR)]
        obuf = qkp.tile([128, 8, T], BF16, name="obuf")
        v_sb = [vtp.tile([128, 8, 65], BF16, name=f"v{c}") for c in range(8)]
        for c in range(8):
            nc.vector.memset(v_sb[c][:, :, 64:65], 1.0)

        # ---- PE warm-up: junk matmuls so HAM un-throttles before real work ----
        warm_ps = scp.tile([128, 2, 512], F32, tag="mm", name="warm_ps")
        for i in range(NWARM):
            nc.tensor.matmul(
                warm_ps[:, i % 2, :], dumm[:, 0:128], dumm[:, :],
                start=True, stop=True, skip_group_check=True)

        # ---- input loads: contiguous partition-major, critical-first ----
        x_sb = wts.tile([128, 8, T], BF16, name="x_sb")
        nc.sync.dma_start(out=x_sb, in_=x_h[:, :, :])
        wq_sb = wts.tile([128, 8, 512], BF16, name="wq_sb")
        nc.scalar.dma_start(out=wq_sb, in_=wq_h[:, :, :])
        wk_sb = wts.tile([128, 8, 512], BF16, name="wk_sb")
        nc.scalar.dma_start(out=wk_sb, in_=wk_h[:, :, :])
        ctx_sb = wts.tile([128, 8, Tc], BF16, name="ctx_sb")
        nc.sync.dma_start(out=ctx_sb, in_=ctx_h[:, :, :])
        wkc_sb = wts.tile([128, 8, 512], BF16, name="wkc_sb")
        nc.sync.dma_start(out=wkc_sb, in_=wkc_h[:, :, :])
        wvc_sb = wts.tile([128, 8, 512], BF16, name="wvc_sb")
        nc.gpsimd.dma_start(out=wvc_sb, in_=wvc_h[:, :, :])
        wv_sb = wts.tile([128, 8, 512], BF16, name="wv_sb")
        nc.gpsimd.dma_start(out=wv_sb, in_=wv_h[:, :, :])
        bvb = consts.tile([128, 512], F32, name="bvb")
        nc.gpsimd.dma_start(out=bvb, in_=_bc0(bv_h[:, :], 128))
        bvcb = consts.tile([128, 512], F32, name="bvcb")
        nc.gpsimd.dma_start(out=bvcb, in_=_bc0(bvc_h[:, :], 128))
        wpj_sb = wts.tile([128, 4, 1024], BF16, name="wpj_sb")
        nc.gpsimd.dma_start(out=wpj_sb, in_=wpj_h[:, :, :])

        # ---- psum-group emitters (each allocates one "mm" tile) ----
        def emit_ctx():
            # kcT: context keys, feature-major, into kT[p][:, 0:Tc]
            for g in range(2):
                pt = scp.tile([128, 2, 512], F32, tag="mm", name=f"pkc{g}")
                for h in range(2):
                    f = 2 * g + h
                    for kc in range(8):
                        nc.tensor.matmul(
                            pt[:, h, 0:Tc], wkc_sb[:, kc, 128 * f:128 * f + 128],
                            ctx_sb[:, kc, :], start=(kc == 0), stop=(kc == 7))
                for h in range(2):
                    f = 2 * g + h
                    nc.vector.tensor_scalar_add(
                        out=kT[f][:, 0:Tc], in0=pt[:, h, 0:Tc],
                        scalar1=bias_kc[:, f:f + 1])

        def qk_filler(p, which, ti):
            w_sb = wq_sb if which == 0 else wk_sb
            dest = qT[p] if which == 0 else kT[p]
            dcol = 0 if which == 0 else Tc
            bcol = p if which == 0 else 4 + p
            ts, te = TCH[ti]

            def f():
                pt = scp.tile([128, 2, 512], F32, tag="mm", name=f"pqk{p}{which}{ti}")
                for kc in range(8):
                    nc.tensor.matmul(
                        pt[:, 0, 0:te - ts], w_sb[:, kc, 128 * p:128 * p + 128],
                        x_sb[:, kc, ts:te], start=(kc == 0), stop=(kc == 7))
                nc.vector.tensor_scalar_add(
                    out=dest[:, dcol + ts:dcol + te], in0=pt[:, 0, 0:te - ts],
                    scalar1=bias_qk[:, bcol:bcol + 1])
            return f

        def emit_vc():
            pt = scp.tile([128, 2, 512], F32, tag="mm", name="pvc")
            for kc in range(8):
                nc.tensor.matmul(
                    pt[:, 0, :], ctx_sb[:, kc, :], wvc_sb[:, kc, :],
                    start=(kc == 0), stop=(kc == 7))
            nc.vector.tensor_add(
                out=v_sb[0][:, :, 0:64],
                in0=pt[:, 0, :].rearrange("p (h d) -> p h d", h=8),
                in1=bvcb.rearrange("p (h d) -> p h d", h=8))

        def emit_v():
            # v natural layout [tok-chunk, head, 64] (+ ones col for denominator)
            for g in range(4):
                tts = [tt for tt in (2 * g, 2 * g + 1) if tt < 7]
                pt = scp.tile([128, 2, 512], F32, tag="mm", name=f"pv{g}")
                for h, tt in enumerate(tts):
                    for kc in range(8):
                        nc.tensor.matmul(
                            pt[:, h, :], x_sb[:, kc, 128 * tt:128 * tt + 128],
                            wv_sb[:, kc, :], start=(kc == 0), stop=(kc == 7))
                for h, tt in enumerate(tts):
                    nc.vector.tensor_add(
                        out=v_sb[1 + tt][:, :, 0:64],
                        in0=pt[:, h, :].rearrange("p (h d) -> p h d", h=8),
                        in1=bvb.rearrange("p (h d) -> p h d", h=8))

        def out_filler(of, ti):
            ts, te = TCH[ti]

            def f():
                pt = scp.tile([128, 2, 512], F32, tag="mm", name=f"po{of}{ti}")
                for kc in range(4):
                    nc.tensor.matmul(
                        pt[:, 0, 0:te - ts], wpj_sb[:, kc, 128 * of:128 * of + 128],
                        aT[kc][:, ts:te], start=(kc == 0), stop=(kc == 3))
                nc.scalar.copy(out=obuf[:, of, ts:te], in_=pt[:, 0, 0:te - ts])
                if ti == 1:
                    nc.sync.dma_start(
                        out=out_h[128 * of:128 * of + 128, :], in_=obuf[:, of, :])
            return f

        fillers = []

        def emit_att(p, ti):
            ts, te = TCH[ti]
            w = te - ts
            lives = [c for c in range(8) if max(128 * (c - 1), ts) < te]
            n = len(lives)
            at = pvp.tile([65, 2, 512], F32, tag="pv", name=f"at{p}{ti}")
            pbs = {}

            def S(c):
                cs = max(128 * (c - 1), ts)
                sc = scp.tile([128, 2, 512], F32, tag="mm", name=f"sc{p}{ti}{c}")
                for hi in range(2):
                    nc.tensor.matmul(
                        sc[:, hi, cs - ts:w],
                        kT[p][64 * hi:64 * hi + 64, 128 * c:128 * c + 128],
                        qT[p][64 * hi:64 * hi + 64, cs:te],
                        start=True, stop=True, tile_position=(64 * hi, 0))
                pb = probsp.tile([128, 2, 512], BF16, tag="pb", name=f"pb{p}{ti}{c}")
                nc.scalar.activation(
                    out=pb[:, :, cs - ts:w], in_=sc[:, :, cs - ts:w], func=AF.Exp,
                    bias=(ebias[:, 1:2] if c == 0 else ebias[:, 0:1]), scale=0.125)
                for s0, e0, mc in _band_pieces(c, ts, te):
                    mb = maskband[:, mc:mc + (e0 - s0)]
                    mb2 = bass.AP(tensor=mb.tensor, offset=mb.offset,
                                  ap=[list(mb.ap[0]), [0, 2], list(mb.ap[1])])
                    nc.vector.tensor_mul(
                        out=pb[:, :, s0 - ts:e0 - ts],
                        in0=pb[:, :, s0 - ts:e0 - ts],
                        in1=mb2)
                pbs[c] = (pb, cs)

            def P(c):
                pb, cs = pbs.pop(c)
                for hi in range(2):
                    nc.tensor.matmul(
                        at[0:65, hi, cs - ts:w], v_sb[c][:, 2 * p + hi, :],
                        pb[:, hi, cs - ts:w],
                        start=(c == lives[0]), stop=(c == lives[-1]),
                        skip_group_check=True)

            si = 0

            def pushS():
                nonlocal si
                if si < n:
                    S(lives[si])
                    si += 1

            pushS()
            pushS()
            for pi in range(n):
                if fillers and pi >= 2:
                    fillers.pop(0)()
                elif si - pi < 5:
                    pushS()  # no filler: deepen score-ahead instead (cap 3)
                pushS()
                P(lives[pi])

            # evacuate PV psum: cast to bf16 staging (DVE), shift to aT via DMA
            stg = smallp.tile([65, 2, 512], BF16, tag="stg", name=f"stg{p}{ti}")
            nc.scalar.copy(out=stg[:, 0, 0:w], in_=at[:, 0, 0:w])
            nc.vector.tensor_copy(out=stg[:, 1, 0:w], in_=at[:, 1, 0:w])
            nc.sync.dma_start(out=aT[p][0:64, ts:te], in_=stg[0:64, 0, 0:w])
            nc.sync.dma_start(out=aT[p][64:128, ts:te], in_=stg[0:64, 1, 0:w])

            def finish():
                # broadcast raw denom row across partitions (K=1 ones-matmul),
                # then 128-lane-parallel fast reciprocal, then normalize aT
                rb = scp.tile([128, 2, 512], F32, tag="mm", name=f"rb{p}{ti}")
                nc.tensor.matmul(
                    rb[0:64, 0, 0:w], ones_sb[64:65, :], stg[64:65, 0, 0:w],
                    start=True, stop=True, tile_position=(64, 0),
                    skip_group_check=True)
                nc.tensor.matmul(
                    rb[64:128, 0, 0:w], ones_sb[64:65, :], stg[64:65, 1, 0:w],
                    start=True, stop=True, tile_position=(64, 64),
                    skip_group_check=True)
                rbc = smallp.tile([128, 512], F32, tag="rbc", name=f"rbc{p}{ti}")
                nc.vector.reciprocal_approx_fast(out=rbc[:, 0:w], in_=rb[:, 0, 0:w])
                nc.vector.tensor_mul(
                    out=aT[p][:, ts:te], in0=aT[p][:, ts:te], in1=rbc[:, 0:w])

            fillers.insert(0, finish)

        # ---- master schedule ----
        for which in range(2):
            for ti in range(2):
                qk_filler(0, which, ti)()
        emit_ctx()
        emit_vc()
        emit_v()
        for p in range(1, NPAIR):
            for which in range(2):
                for ti in range(2):
                    fillers.append(qk_filler(p, which, ti))

        emit_att(0, 0)
        emit_att(0, 1)
        emit_att(1, 0)
        emit_att(1, 1)
        emit_att(2, 0)
        emit_att(2, 1)
        emit_att(3, 0)
        # att(3,0)'s finish (normalizes aT[3][:, t0]) must precede out-t0 reads
        fillers.extend(out_filler(of, 0) for of in range(8))
        emit_att(3, 1)
        fillers.extend(out_filler(of, 1) for of in range(8))
        while fillers:
            fillers.pop(0)()

    if not nc.is_finalized():
        nc.finalize()
    return nc


_NC_CACHE = {}


def _get_nc():
    if "nc" not in _NC_CACHE:
        _NC_CACHE["nc"] = build_nc()
    return _NC_CACHE["nc"]


def _pack128(v):
    """[128*n] -> [128, n] with [p, f] = v[128*f + p]."""
    n = v.shape[0] // 128
    return np.ascontiguousarray(v.reshape(n, 128).T)


def make_in_maps(inputs):
    bf16 = ml_dtypes.bfloat16
    x = np.asarray(inputs["x"], np.float32)
    ctx_seq = np.asarray(inputs["context_seq"], np.float32)
    w_ref = np.asarray(inputs["w_ref"], np.float32)
    b_ref = np.asarray(inputs["b_ref"], np.float32)
    w_attn = np.asarray(inputs["w_attn"], np.float32)
    b_attn = np.asarray(inputs["b_attn"], np.float32)
    w_proj = np.asarray(inputs["w_proj"], np.float32)

    # mask band constant: cols 0-127 causal (1 where q>=p), cols 128-255
    # anti-diagonal (0 where q==p else 1)
    qq = np.arange(128)[None, :]
    pp = np.arange(128)[:, None]
    mband = np.concatenate([(qq >= pp), (qq != pp)], axis=1).astype(bf16)
    mband = np.ascontiguousarray(mband)

    def _pm(a, nkc=8):
        # [nkc*128, cols] -> partition-major [128, nkc, cols]
        return np.ascontiguousarray(
            a.reshape(nkc, 128, a.shape[1]).transpose(1, 0, 2))

    in_maps = []
    for b in range(4):
        xT = _pm(x[b].T.astype(bf16))
        ctxT = _pm(ctx_seq[b].T.astype(bf16))
        for g in range(2):
            sl = slice(512 * g, 512 * g + 512)
            in_maps.append(dict(
                xT=xT,
                ctxT=ctxT,
                w_q=_pm(w_attn[:, 0 * NX:1 * NX][:, sl].astype(bf16)),
                w_k=_pm(w_attn[:, 1 * NX:2 * NX][:, sl].astype(bf16)),
                w_v=_pm(w_attn[:, 2 * NX:3 * NX][:, sl].astype(bf16)),
                w_kc=_pm(w_ref[:, 0 * NX:1 * NX][:, sl].astype(bf16)),
                w_vc=_pm(w_ref[:, 1 * NX:2 * NX][:, sl].astype(bf16)),
                w_pj=_pm(w_proj[sl, :].astype(bf16), nkc=4),
                b_qk=_pack128(np.concatenate([b_attn[0 * NX:1 * NX][sl],
                                              b_attn[1 * NX:2 * NX][sl]])),
                b_kc=_pack128(b_ref[0 * NX:1 * NX][sl]),
                b_v=np.ascontiguousarray(b_attn[2 * NX:3 * NX][sl].reshape(1, 512)),
                b_vc=np.ascontiguousarray(b_ref[1 * NX:2 * NX][sl].reshape(1, 512)),
                mband=mband,
            ))
    return in_maps


def kernel(**inputs):
    b_proj = np.asarray(inputs["b_proj"], np.float32)
    in_maps = make_in_maps(inputs)
    nc = _get_nc()
    res = run_bass_kernel_spmd(nc, in_maps, core_ids=list(range(8)),
                               trace=os.environ.get("COCON_TRACE", "") == "1")
    outs = res.results
    out = np.empty((4, T, NX), np.float32)
    for b in range(4):
        acc = outs[2 * b]["outT"].astype(np.float32) + \
            outs[2 * b + 1]["outT"].astype(np.float32)  # [1024, 896]
        out[b] = acc.T + b_proj[None, :]
    if res.exec_time_ns is not None:
        kernel.last_exec_time_ns = res.exec_time_ns
    return out


kernel.last_exec_time_ns = None
